# revision 6
# baseline (speedup 1.0000x reference)
"""Multi-head cross-attention (B=4, S=2048, D=1024, H=16) on 8 Trainium2 cores.

Sharding: hybrid data/tensor parallel. Core c handles batch b = c//2 and
head-group g = c%2 (8 of the 16 heads, i.e. 512 of the 1024 q/k/v dims).
Each core computes a partial out-projection over its 512 attention dims;
the host sums the two partials per batch.

Design (v3):
- ACT engine runs ONLY exp; its ~285us busy is the kernel floor. The
  key-padding mask is applied by zeroing masked keys' V rows and ones
  column (exactly equivalent to -inf logits), so one biasless exp spans
  two key chunks ([128,1024]).
- S=K.T@Q contracts over head_dim=64: issued as PE row-tiled pairs
  (tile_position (0,0)/(64,0)) emitted adjacently so both heads stream
  concurrently on the two array halves.
- Per query block (512 queries) the work is two phases: A = S+exp for
  all 16 key chunks (es kept in a 16-deep SBUF ring), B = the 32 AV
  accumulation matmuls. A(g+1) is emitted interleaved into B(g) so the
  exp stream never waits on AV/normalize; softmax normalize runs off
  the critical path during the next block's A phase.
- All projections (K1-3, Q1-3, V, O) are pumped as small filler batches
  inside A/B so the PE stays dense (HAM stays at 2.4GHz) and projection
  time hides entirely under the exp-bound attention span.
- PSUM: lg_e(2) lg_o(2) av_e(1) av_o(1) pj(2) = 8 banks.

bv is folded into bo on the host (softmax rows sum to 1).
"""

import numpy as np

import concourse.bacc as bacc
import concourse.mybir as mybir
from concourse import tile
from concourse.bass_utils import run_bass_kernel_spmd

F32 = mybir.dt.float32
F16 = mybir.dt.float16
AF = mybir.ActivationFunctionType

B, S, D = 4, 2048, 1024
H, HD = 16, 64
NCORES = 8
NH = 8          # heads per core
OD = NH * HD    # 512 attention dims per core
P = 128
NDC = D // P    # 8 d-chunks
NKC = S // P    # 16 key chunks
NMT = OD // P   # 4 head-pairs
NG = NMT * 4    # 16 query-block groups

_cache = {}


def _build():
    from collections import deque
    from contextlib import ExitStack

    nc = bacc.Bacc(None, target_bir_lowering=False, debug=False)

    x_t = nc.dram_tensor("x_t", [D, S], F16, kind="ExternalInput").ap()
    mem_t = nc.dram_tensor("mem_t", [D, S], F16, kind="ExternalInput").ap()
    wq_t = nc.dram_tensor("wq_t", [D, OD], F16, kind="ExternalInput").ap()
    wk_t = nc.dram_tensor("wk_t", [D, OD], F16, kind="ExternalInput").ap()
    wv_t = nc.dram_tensor("wv_t", [D, OD], F16, kind="ExternalInput").ap()
    wo_t = nc.dram_tensor("wo_t", [OD, D], F16, kind="ExternalInput").ap()
    bq_s = nc.dram_tensor("bq_s", [P, OD // P], F32, kind="ExternalInput").ap()
    bk_s = nc.dram_tensor("bk_s", [P, OD // P], F32, kind="ExternalInput").ap()
    bo_s = nc.dram_tensor("bo_s", [P, D // P], F32, kind="ExternalInput").ap()
    vmask = nc.dram_tensor("vmask", [P, NKC], F32, kind="ExternalInput").ap()
    vmask8 = nc.dram_tensor("vmask8", [P, NKC * NH], F16,
                            kind="ExternalInput").ap()
    out_t = nc.dram_tensor("out_t", [D, S], F16, kind="ExternalOutput").ap()

    x_c = x_t.rearrange("(c p) s -> c p s", p=P)
    m_c = mem_t.rearrange("(c p) s -> c p s", p=P)
    wq_c = wq_t.rearrange("(c p) o -> c p o", p=P)
    wk_c = wk_t.rearrange("(c p) o -> c p o", p=P)
    wv_c = wv_t.rearrange("(c p) o -> c p o", p=P)
    wo_c = wo_t.rearrange("(c p) o -> c p o", p=P)

    with tile.TileContext(nc) as tc, ExitStack() as ctx:
        q_pool = ctx.enter_context(tc.tile_pool(name="qt", bufs=1))
        k_pool = ctx.enter_context(tc.tile_pool(name="kt", bufs=1))
        v_pool = ctx.enter_context(tc.tile_pool(name="va", bufs=1))
        a_pool = ctx.enter_context(tc.tile_pool(name="at", bufs=1))
        c_pool = ctx.enter_context(tc.tile_pool(name="cst", bufs=1))
        w_pool = ctx.enter_context(tc.tile_pool(name="wt", bufs=10))
        e_pool = ctx.enter_context(tc.tile_pool(name="es", bufs=16))
        n_pool = ctx.enter_context(tc.tile_pool(name="nrm", bufs=2))
        o_pool = ctx.enter_context(tc.tile_pool(name="ev", bufs=3))
        psum_pool = ctx.enter_context(tc.tile_pool(name="ps", bufs=1, space="PSUM"))
        m_pool = ctx.enter_context(tc.tile_pool(name="mm", bufs=8))
        x_pool = ctx.enter_context(tc.tile_pool(name="xx", bufs=8))

        # ---- constants ----
        bq_sb = c_pool.tile([P, OD // P], F32, tag="bq")
        bk_sb = c_pool.tile([P, OD // P], F32, tag="bk")
        bo_sb = c_pool.tile([P, D // P], F32, tag="bo")
        vm_sb = c_pool.tile([P, NKC], F32, tag="vm")
        vm8_sb = c_pool.tile([P, NKC, NH], F16, tag="vm8")
        nc.sync.dma_start(out=bq_sb[:], in_=bq_s[:])
        nc.sync.dma_start(out=bk_sb[:], in_=bk_s[:])
        nc.sync.dma_start(out=bo_sb[:], in_=bo_s[:])
        nc.sync.dma_start(out=vm_sb[:], in_=vmask[:])
        nc.sync.dma_start(
            out=vm8_sb[:], in_=vmask8.rearrange("p (s h) -> p s h", h=NH))

        # ---- weight DMAs first (small, needed first), then bulk inputs:
        # memory on both queues (K0/V gate on it), then x (Q0 gates on it)
        w0_tiles = {"k": [], "q": [], "v": []}
        for i in range(NDC):
            wt = w_pool.tile([P, P], F16, tag="w", name="wk0", bufs=10)
            nc.sync.dma_start(out=wt[:], in_=wk_c[i, :, 0:P])
            w0_tiles["k"].append(wt)
        for i in range(NDC):
            wt = w_pool.tile([P, P], F16, tag="wq0", name="wq0", bufs=8)
            nc.gpsimd.dma_start(out=wt[:], in_=wq_c[i, :, 0:P])
            w0_tiles["q"].append(wt)
        # m/x as column blocks [128, 512]: block cb holds tokens/queries
        # [cb*512, (cb+1)*512) so compute can start after ~2MB instead of 8.5
        m_cb = [[None] * NDC for _ in range(4)]
        x_cb = [[None] * NDC for _ in range(4)]

        QS = [nc.sync, nc.scalar]

        def load_mcb(cb):
            for i in range(NDC):
                t = m_pool.tile([P, 512], F16, tag=f"m{cb}", name="mt")
                QS[i % 2].dma_start(
                    out=t[:], in_=m_c[i, :, cb * 512:(cb + 1) * 512])
                m_cb[cb][i] = t

        def load_xcb(cb):
            for i in range(NDC):
                t = x_pool.tile([P, 512], F16, tag=f"x{cb}", name="xt")
                QS[(i + 1) % 2].dma_start(
                    out=t[:], in_=x_c[i, :, cb * 512:(cb + 1) * 512])
                x_cb[cb][i] = t
        load_mcb(0)
        load_xcb(0)
        for i in range(NDC):
            wt = w_pool.tile([P, OD], F16, tag="wv", name="wvt", bufs=8)
            QS[i % 2].dma_start(out=wt[:], in_=wv_c[i])
            w0_tiles["v"].append(wt)
        for cb in (1, 2, 3):
            load_mcb(cb)
        load_xcb(1)

        # ---- persistent tiles ----
        qT = [q_pool.tile([P, S], F16, tag=f"q{m}", name=f"q{m}")
              for m in range(NMT)]
        # kT packs a head pair: partitions 0:64 = head 2m, 64:128 = head 2m+1
        kT = [k_pool.tile([P, S], F16, tag=f"k{m}", name=f"k{m}")
              for m in range(NMT)]
        v_aug = [v_pool.tile([P, NH, 65], F16, tag=f"v{st}", name=f"v{st}")
                 for st in range(NKC)]
        attn = [a_pool.tile([P, S], F16, tag=f"a{m}", name=f"a{m}")
                for m in range(NMT)]

        def kq_proj_steps(wc, src_cb, dst, bias, m, w_tiles=None,
                          halves=range(4)):
            if w_tiles is None:
                w_tiles = []

                def load_w():
                    for i in range(NDC):
                        wt = w_pool.tile([P, P], F16, tag="w", name="wkq",
                                         bufs=10)
                        nc.sync.dma_start(
                            out=wt[:], in_=wc[i, :, m * P:(m + 1) * P])
                        w_tiles.append(wt)
                steps = [load_w]
            else:
                steps = []
            for half in halves:
                csl = slice(half * 512, (half + 1) * 512)
                ps = []

                def mm(i, ps=ps, half=half):
                    if i == 0:
                        ps.append(psum_pool.tile([P, 512], F32, tag="pj",
                                                 name="pskq", bufs=2))
                    nc.tensor.matmul(
                        ps[0][:], w_tiles[i][:], src_cb[half][i][:],
                        start=(i == 0), stop=(i == NDC - 1))
                for i in range(NDC):
                    steps.append(lambda i=i, mm=mm: mm(i))

                def evac(ps=ps, csl=csl):
                    nc.vector.tensor_scalar_add(
                        dst[:, csl], ps[0][:], bias[:, m:m + 1])
                steps.append(evac)
            return steps

        # V-proj: per-token-chunk projection (preloaded weights)
        wv_tiles = w0_tiles["v"]

        def v_proj_steps(st):
            ps = []

            def mm(i):
                if i == 0:
                    ps.append(psum_pool.tile([P, 512], F32, tag="pj",
                                             name="psv", bufs=2))
                nc.tensor.matmul(
                    ps[0][:],
                    m_cb[st // 4][i][:, (st % 4) * P:(st % 4 + 1) * P],
                    wv_tiles[i][:],
                    start=(i == 0), stop=(i == NDC - 1))
            steps = [lambda i=i, mm=mm: mm(i) for i in range(NDC)]

            def evac():
                nc.vector.tensor_scalar_mul(
                    v_aug[st][:, 0:NH, 0:64],
                    ps[0][:].rearrange("p (h d) -> p h d", h=NH),
                    vm_sb[:, st:st + 1])
                nc.gpsimd.tensor_copy(
                    v_aug[st][:, 0:NH, 64:65], vm8_sb[:, st, :].unsqueeze(2))
            steps.append(evac)
            return steps

        def v_proj(st):
            for step in v_proj_steps(st):
                step()

        wo_tiles = [[None] * NMT for _ in range(D // P)]

        def load_wo(m):
            for i in range(NMT):
                wt = w_pool.tile([P, P], F16, tag="wo", name="wot", bufs=32)
                nc.sync.dma_start(out=wt[:], in_=wo_c[i, :, m * P:(m + 1) * P])
                wo_tiles[m][i] = wt

        def o_proj_steps(m, jb):
            ps = []

            def mm(i):
                if i == 0:
                    ps.append(psum_pool.tile([P, 512], F32, tag="pj",
                                             name="pso", bufs=2))
                nc.tensor.matmul(
                    ps[0][:], wo_tiles[m][i][:],
                    attn[i][:, jb * 512:(jb + 1) * 512],
                    start=(i == 0), stop=(i == NMT - 1))
            steps = [lambda i=i, mm=mm: mm(i) for i in range(NMT)]

            def evac():
                ev = o_pool.tile([P, 512], F16, tag="ev")
                nc.vector.tensor_scalar_add(ev[:], ps[0][:], bo_sb[:, m:m + 1])
                nc.sync.dma_start(
                    out=out_t[m * P:(m + 1) * P, jb * 512:(jb + 1) * 512],
                    in_=ev[:])
            steps.append(evac)
            return steps

        # Q weights for all head-pairs preloaded (tiny); halves emitted
        # on demand at loop tops
        wq_all = [w0_tiles["q"]]
        for m in (1, 2, 3):
            tiles = []
            for i in range(NDC):
                wt = w_pool.tile([P, P], F16, tag=f"wq{m}", name="wqm", bufs=8)
                nc.sync.dma_start(out=wt[:], in_=wq_c[i, :, m * P:(m + 1) * P])
                tiles.append(wt)
            wq_all.append(tiles)
        for cb in (2, 3):
            load_xcb(cb)

        # ---- attention pipeline ----
        esbuf = {}
        avbuf = {}

        def emit_A(g, k2):
            mt, qb = divmod(g, 4)
            qsl = slice(qb * 512, (qb + 1) * 512)
            ka, kb = 2 * k2, 2 * k2 + 1
            lg_e = psum_pool.tile([P, 1024], F32, tag="lg", name="lg_e",
                                  bufs=2)
            lg_o = psum_pool.tile([P, 1024], F32, tag="lg", name="lg_o",
                                  bufs=2)
            # same-row-group matmuls adjacent: each PE array-config switch
            # (h0<->h64<->full) costs ~86ns of issue stall, so h0,h0,h64,h64
            # is two switches cheaper than h0,h64,h0,h64
            for half, kc in ((0, ka), (1, kb)):
                nc.tensor.matmul(
                    lg_e[:, half * 512:(half + 1) * 512],
                    kT[mt][0:64, kc * P:(kc + 1) * P],
                    qT[mt][0:64, qsl], start=True, stop=True)
            for half, kc in ((0, ka), (1, kb)):
                nc.tensor.matmul(
                    lg_o[:, half * 512:(half + 1) * 512],
                    kT[mt][64:128, kc * P:(kc + 1) * P],
                    qT[mt][64:128, qsl], start=True, stop=True)
            es_e = e_pool.tile([P, 1024], F16, tag="es", bufs=16)
            nc.scalar.activation(es_e[:], lg_e[:], AF.Exp, scale=0.125)
            es_o = e_pool.tile([P, 1024], F16, tag="es", bufs=16)
            nc.scalar.activation(es_o[:], lg_o[:], AF.Exp, scale=0.125)
            esbuf[(g, k2)] = (es_e, es_o)

        def emit_B(g, k2):
            mt, qb = divmod(g, 4)
            he, ho = 2 * mt, 2 * mt + 1
            if k2 == 0:
                avbuf[g] = (
                    psum_pool.tile([P, 512], F32, tag="av_e", name="av_e"),
                    psum_pool.tile([P, 512], F32, tag="av_o", name="av_o"))
            av_e, av_o = avbuf[g]

            es_e, es_o = esbuf.pop((g, k2))
            ka, kb = 2 * k2, 2 * k2 + 1
            for half, kc in ((0, ka), (1, kb)):
                va = v_aug[kc][:].rearrange("p h d -> p (h d)")
                nc.tensor.matmul(
                    av_e[0:65, :], va[:, 65 * he:65 * he + 65],
                    es_e[:, half * 512:(half + 1) * 512],
                    start=(k2 == 0 and half == 0),
                    stop=(k2 == NKC // 2 - 1 and half == 1))
            for half, kc in ((0, ka), (1, kb)):
                va = v_aug[kc][:].rearrange("p h d -> p (h d)")
                nc.tensor.matmul(
                    av_o[0:65, :], va[:, 65 * ho:65 * ho + 65],
                    es_o[:, half * 512:(half + 1) * 512],
                    start=(k2 == 0 and half == 0),
                    stop=(k2 == NKC // 2 - 1 and half == 1))


        def emit_norm(g):
            mt, qb = divmod(g, 4)
            qsl = slice(qb * 512, (qb + 1) * 512)
            av_e, av_o = avbuf.pop(g)
            for ro, av in ((0, av_e), (64, av_o)):
                dn = n_pool.tile([1, 512], F32, tag="dn")
                r0 = n_pool.tile([1, 512], F32, tag="r0")
                r0b = n_pool.tile([1, 512], F32, tag="r0b")
                bc = n_pool.tile([64, 512], F32, tag="bc")
                nc.vector.tensor_copy(dn[:], av[64:65, :])
                # custom-DVE op; the tensor_copy after it (same DVE FIFO)
                # bridges its result into tracked dependencies for gpsimd
                nc.vector.reciprocal_approx_fast(out=r0[:], in_=dn[:])
                nc.vector.tensor_copy(r0b[:], r0[:])
                nc.gpsimd.partition_broadcast(bc[:], r0b[:])
                nc.vector.tensor_mul(
                    attn[mt][ro:ro + 64, qsl], av[0:64, :], bc[:])

        # ---- deadline-ordered unit scheduler ----
        # Emission slot t: A(g) occupies slots [SA(g), SA(g)+8) where
        # SA(0)=0, SA(1)=8, SA(g)=16+8(g-2) for g>=2 (A(g) is emitted two
        # phases ahead of B(g), which shares slots with A(g+2)). Every
        # projection is chopped into ~0.3us units with a deadline slot;
        # the drain interleaves a few units per slot so no PE bulge ever
        # exceeds the exp-cadence slack (the lg ring buffers only one k2).
        units = deque()

        def add_units(steps, dl):
            n = len(steps)
            for i, s in enumerate(steps):
                units.append((dl - (n - 1 - i) // 3, s))

        def drain(t, eager=3):
            n = 0
            while units and (units[0][0] <= t or n < eager):
                if units[0][0] > t:
                    n += 1
                units.popleft()[1]()

        def sa(g):
            return 16 + 8 * (g - 2)

        # g = mt*4 + qb (mt slowest): kT[m] is first needed by A(4m) at
        # slot sa(4m), so K1/K2/K3 projections spread across the whole
        # timeline instead of bunching in slots 15-26. V is needed from
        # B(0) (slots 16-23) and stays early. All units queued in
        # ascending-deadline order (the drain is FIFO).
        k0 = kq_proj_steps(wk_c, m_cb, kT[0], bk_sb, 0, w_tiles=w0_tiles["k"])
        q0 = kq_proj_steps(wq_c, x_cb, qT[0], bq_sb, 0, w_tiles=w0_tiles["q"],
                           halves=[0])
        vs = [v_proj_steps(st) for st in range(NKC)]
        k1 = kq_proj_steps(wk_c, m_cb, kT[1], bk_sb, 1)
        k2s = kq_proj_steps(wk_c, m_cb, kT[2], bk_sb, 2)
        k3s = kq_proj_steps(wk_c, m_cb, kT[3], bk_sb, 3)

        def q_half(mt_, qb_):
            return kq_proj_steps(wq_c, x_cb, qT[mt_], bq_sb, mt_,
                                 w_tiles=wq_all[mt_], halves=[qb_])

        sched = [(0, k0[0:9]), (0, q0), (1, k0[9:18]),
                 (3, k0[18:27]), (5, k0[27:36])]
        for st in range(NKC):
            sched.append((3 + (st * 11) // 16, vs[st]))
        for qb_ in (1, 2, 3):
            sched.append((8 * qb_ - 2, q_half(0, qb_)))
        for m_, ks in ((1, k1), (2, k2s), (3, k3s)):
            base = sa(4 * m_)
            sched.append((base - 7, ks[0:10]))
            sched.append((base - 4, ks[10:19]))
            sched.append((base - 2, ks[19:28]))
            sched.append((base, ks[28:37]))
        for g in range(4, NG):
            mt_, qb_ = divmod(g, 4)
            sched.append((sa(g) - 2, q_half(mt_, qb_)))
        for dl, steps in sorted(sched, key=lambda p: p[0]):
            add_units(steps, dl)

        for k2 in range(NKC // 2):
            drain(k2)
            emit_A(0, k2)
        for k2 in range(NKC // 2):
            drain(8 + k2)
            emit_A(1, k2)
        for g in range(NG):
            for k2 in range(NKC // 2):
                drain(sa(g + 2) + k2)
                emit_B(g, k2)
                if g + 2 < NG:
                    emit_A(g + 2, k2)
            emit_norm(g)
            mt, qb = divmod(g, 4)
            if mt == NMT - 1 and qb < 3:
                for m in range(D // P):
                    if qb == 0:
                        units.append((10 ** 9, lambda m=m: load_wo(m)))
                    add_units(o_proj_steps(m, qb), 10 ** 9)

        # ---- drain remaining units, then O-proj for the last column ----
        while units:
            units.popleft()[1]()
        for m in range(D // P):
            for step in o_proj_steps(m, 3):
                step()

    nc.compile()
    return nc


def _prep_inputs(x, memory, mask, wq, bq, wk, bk, wv, bv, wo, bo):
    f = np.float32
    h = np.float16
    wqT = np.ascontiguousarray(wq.T, dtype=f)
    wkT = np.ascontiguousarray(wk.T, dtype=f)
    wvT = np.ascontiguousarray(wv.T, dtype=f)
    woT = np.ascontiguousarray(wo.T, dtype=f)
    bo_eff = (bo.astype(f) + wo.astype(f) @ bv.astype(f))
    zeros_bo = np.zeros_like(bo_eff)
    in_maps = []
    for c in range(NCORES):
        b, g = divmod(c, 2)
        sl = slice(g * OD, (g + 1) * OD)
        bo_c = bo_eff if g == 0 else zeros_bo
        vm = np.where(mask[b], np.float32(0.0), np.float32(1.0)).astype(f)
        vm_s = np.ascontiguousarray(vm.reshape(NKC, P).T)      # [P, NKC]
        vm8 = np.repeat(vm_s.astype(h)[:, :, None], NH, axis=2)  # [P,NKC,NH]
        in_maps.append({
            "x_t": np.ascontiguousarray(x[b].T, dtype=h),
            "mem_t": np.ascontiguousarray(memory[b].T, dtype=h),
            "wq_t": np.ascontiguousarray(wqT[:, sl]).astype(h),
            "wk_t": np.ascontiguousarray(wkT[:, sl]).astype(h),
            "wv_t": np.ascontiguousarray(wvT[:, sl]).astype(h),
            "wo_t": np.ascontiguousarray(woT[sl, :]).astype(h),
            "bq_s": np.ascontiguousarray(bq[sl].astype(f).reshape(OD // P, P).T),
            "bk_s": np.ascontiguousarray(bk[sl].astype(f).reshape(OD // P, P).T),
            "bo_s": np.ascontiguousarray(bo_c.reshape(D // P, P).T),
            "vmask": vm_s,
            "vmask8": np.ascontiguousarray(vm8.reshape(P, NKC * NH)),
        })
    return in_maps


def kernel(x, memory, mask, wq, bq, wk, bk, wv, bv, wo, bo, **run_kwargs):
    x = np.asarray(x, dtype=np.float32)
    memory = np.asarray(memory, dtype=np.float32)
    mask = np.asarray(mask)
    if "nc" not in _cache:
        _cache["nc"] = _build()
    nc = _cache["nc"]
    in_maps = _prep_inputs(x, memory, mask, wq, bq, wk, bk, wv, bv, wo, bo)
    res = run_bass_kernel_spmd(nc, in_maps, list(range(NCORES)), **run_kwargs)
    out = np.empty((B, S, D), dtype=np.float32)
    for b in range(B):
        part = (res.results[2 * b]["out_t"].astype(np.float32)
                + res.results[2 * b + 1]["out_t"].astype(np.float32))
        out[b] = part.T
    if run_kwargs:
        _cache["last_results"] = res
    return out



# revision 23
# speedup vs baseline: 1.0274x; 1.0274x over previous
"""Multi-head cross-attention (B=4, S=2048, D=1024, H=16) on 8 Trainium2 cores.

Sharding: hybrid data/tensor parallel. Core c handles batch b = c//2 and
head-group g = c%2 (8 of the 16 heads, i.e. 512 of the 1024 q/k/v dims).
Each core computes a partial out-projection over its 512 attention dims;
the host sums the two partials per batch.

Design (v3):
- ACT engine runs ONLY exp; its ~285us busy is the kernel floor. The
  key-padding mask is applied by zeroing masked keys' V rows and ones
  column (exactly equivalent to -inf logits), so one biasless exp spans
  two key chunks ([128,1024]).
- S=K.T@Q contracts over head_dim=64: issued as PE row-tiled pairs
  (tile_position (0,0)/(64,0)) emitted adjacently so both heads stream
  concurrently on the two array halves.
- Per query block (512 queries) the work is two phases: A = S+exp for
  all 16 key chunks (es kept in a 16-deep SBUF ring), B = the 32 AV
  accumulation matmuls. A(g+1) is emitted interleaved into B(g) so the
  exp stream never waits on AV/normalize; softmax normalize runs off
  the critical path during the next block's A phase.
- All projections (K1-3, Q1-3, V, O) are pumped as small filler batches
  inside A/B so the PE stays dense (HAM stays at 2.4GHz) and projection
  time hides entirely under the exp-bound attention span.
- PSUM: lg_e(2) lg_o(2) av_e(1) av_o(1) pj(2) = 8 banks.

bv is folded into bo on the host (softmax rows sum to 1).
"""

import numpy as np

import concourse.bacc as bacc
import concourse.mybir as mybir
from concourse import tile
from concourse.bass_utils import run_bass_kernel_spmd

F32 = mybir.dt.float32
F16 = mybir.dt.float16
AF = mybir.ActivationFunctionType

B, S, D = 4, 2048, 1024
H, HD = 16, 64
NCORES = 8
NH = 8          # heads per core
OD = NH * HD    # 512 attention dims per core
P = 128
NDC = D // P    # 8 d-chunks
NKC = S // P    # 16 key chunks
NMT = OD // P   # 4 head-pairs
NG = NMT * 4    # 16 query-block groups

_cache = {}


def _build():
    from collections import deque
    from contextlib import ExitStack

    nc = bacc.Bacc(None, target_bir_lowering=False, debug=False)

    # All bulk inputs are packed host-side as exact SBUF images
    # ([partition, free...] with per-partition-contiguous free bytes) so
    # each block loads with one dma_start and 8KB-run descriptors.
    x_t = nc.dram_tensor("x_t", [4, P, NDC, 512], F16,
                         kind="ExternalInput").ap()
    mem_t = nc.dram_tensor("mem_t", [4, P, NDC, 512], F16,
                           kind="ExternalInput").ap()
    wq_t = nc.dram_tensor("wq_t", [NMT, P, NDC, P], F16,
                          kind="ExternalInput").ap()
    wk_t = nc.dram_tensor("wk_t", [NMT, P, NDC, P], F16,
                          kind="ExternalInput").ap()
    wv_t = nc.dram_tensor("wv_t", [P, NDC, OD], F16,
                          kind="ExternalInput").ap()
    wo_t = nc.dram_tensor("wo_t", [D // P, P, NMT, P], F16,
                          kind="ExternalInput").ap()
    bq_s = nc.dram_tensor("bq_s", [P, OD // P], F32, kind="ExternalInput").ap()
    bk_s = nc.dram_tensor("bk_s", [P, OD // P], F32, kind="ExternalInput").ap()
    bo_s = nc.dram_tensor("bo_s", [P, D // P], F32, kind="ExternalInput").ap()
    vmask = nc.dram_tensor("vmask", [P, NKC], F32, kind="ExternalInput").ap()
    vmask8 = nc.dram_tensor("vmask8", [P, NKC * NH], F16,
                            kind="ExternalInput").ap()
    out_t = nc.dram_tensor("out_t", [D, S], F16, kind="ExternalOutput").ap()

    with tile.TileContext(nc) as tc, ExitStack() as ctx:
        q_pool = ctx.enter_context(tc.tile_pool(name="qt", bufs=1))
        k_pool = ctx.enter_context(tc.tile_pool(name="kt", bufs=1))
        v_pool = ctx.enter_context(tc.tile_pool(name="va", bufs=1))
        a_pool = ctx.enter_context(tc.tile_pool(name="at", bufs=1))
        c_pool = ctx.enter_context(tc.tile_pool(name="cst", bufs=1))
        w_pool = ctx.enter_context(tc.tile_pool(name="wt", bufs=10))
        e_pool = ctx.enter_context(tc.tile_pool(name="es", bufs=15))
        n_pool = ctx.enter_context(tc.tile_pool(name="nrm", bufs=2))
        o_pool = ctx.enter_context(tc.tile_pool(name="ev", bufs=2))
        psum_pool = ctx.enter_context(tc.tile_pool(name="ps", bufs=1, space="PSUM"))
        m_pool = ctx.enter_context(tc.tile_pool(name="mm", bufs=8))
        x_pool = ctx.enter_context(tc.tile_pool(name="xx", bufs=8))

        # ---- constants ----
        bq_sb = c_pool.tile([P, OD // P], F32, tag="bq")
        bk_sb = c_pool.tile([P, OD // P], F32, tag="bk")
        bo_sb = c_pool.tile([P, D // P], F32, tag="bo")
        vm_sb = c_pool.tile([P, NKC], F32, tag="vm")
        vm8_sb = c_pool.tile([P, NKC, NH], F16, tag="vm8")
        nc.sync.dma_start(out=bq_sb[:], in_=bq_s[:])
        nc.sync.dma_start(out=bk_sb[:], in_=bk_s[:])
        nc.sync.dma_start(out=bo_sb[:], in_=bo_s[:])
        nc.sync.dma_start(out=vm_sb[:], in_=vmask[:])
        nc.sync.dma_start(
            out=vm8_sb[:], in_=vmask8.rearrange("p (s h) -> p s h", h=NH))

        # ---- weight DMAs first (small, needed first), then bulk inputs:
        # memory on both queues (K0/V gate on it), then x (Q0 gates on it).
        # Each block is one SBUF-image dma_start; block 0 of m/x is split
        # across two queues so the first K/Q matmuls can start earliest.
        wk0_blk = w_pool.tile([P, NDC, P], F16, tag="wk0", name="wk0", bufs=1)
        nc.sync.dma_start(out=wk0_blk[:], in_=wk_t[0])
        wq0_blk = w_pool.tile([P, NDC, P], F16, tag="wq0", name="wq0", bufs=1)
        nc.gpsimd.dma_start(out=wq0_blk[:], in_=wq_t[0])
        w0_tiles = {
            "k": [wk0_blk[:, i, :] for i in range(NDC)],
            "q": [wq0_blk[:, i, :] for i in range(NDC)],
        }
        m_cb = [None] * 4
        x_cb = [None] * 4

        QS = [nc.sync, nc.scalar]

        def load_blk(cb, src, pool, tag, split):
            t = pool.tile([P, NDC, 512], F16, tag=tag, name=tag, bufs=1)
            if split:
                QS[0].dma_start(out=t[:, 0:4, :], in_=src[cb, :, 0:4, :])
                QS[1].dma_start(out=t[:, 4:8, :], in_=src[cb, :, 4:8, :])
            else:
                QS[cb % 2].dma_start(out=t[:], in_=src[cb])
            return [t[:, i, :] for i in range(NDC)]

        m_cb[0] = load_blk(0, mem_t, m_pool, "m0", True)
        x_cb[0] = load_blk(0, x_t, x_pool, "x0", True)
        wv_blk = w_pool.tile([P, NDC, OD], F16, tag="wv", name="wvt", bufs=1)
        nc.gpsimd.dma_start(out=wv_blk[:], in_=wv_t[:])
        w0_tiles["v"] = [wv_blk[:, i, :] for i in range(NDC)]
        for cb in (1, 2, 3):
            m_cb[cb] = load_blk(cb, mem_t, m_pool, f"m{cb}", False)
        x_cb[1] = load_blk(1, x_t, x_pool, "x1", False)

        # ---- persistent tiles ----
        qT = [q_pool.tile([P, S], F16, tag=f"q{m}", name=f"q{m}")
              for m in range(NMT)]
        # kT packs a head pair: partitions 0:64 = head 2m, 64:128 = head 2m+1
        kT = [k_pool.tile([P, S], F16, tag=f"k{m}", name=f"k{m}")
              for m in range(NMT)]
        v_aug = [v_pool.tile([P, NH, 65], F16, tag=f"v{st}", name=f"v{st}")
                 for st in range(NKC)]
        attn = [a_pool.tile([P, S], F16, tag=f"a{m}", name=f"a{m}")
                for m in range(NMT)]

        def kq_proj_steps(wc, src_cb, dst, bias, m, w_tiles=None,
                          halves=range(4)):
            if w_tiles is None:
                w_tiles = []

                def load_w():
                    blk = w_pool.tile([P, NDC, P], F16, tag="w", name="wkq",
                                      bufs=2)
                    nc.sync.dma_start(out=blk[:], in_=wc[m])
                    w_tiles.extend(blk[:, i, :] for i in range(NDC))
                steps = [load_w]
            else:
                steps = []
            for half in halves:
                csl = slice(half * 512, (half + 1) * 512)
                ps = []

                def mm(i, ps=ps, half=half):
                    if i == 0:
                        ps.append(psum_pool.tile([P, 512], F32, tag="pj",
                                                 name="pskq", bufs=2))
                    nc.tensor.matmul(
                        ps[0][:], w_tiles[i][:], src_cb[half][i][:],
                        start=(i == 0), stop=(i == NDC - 1))
                for i in range(NDC):
                    steps.append(lambda i=i, mm=mm: mm(i))

                def evac(ps=ps, csl=csl):
                    nc.vector.tensor_scalar_add(
                        dst[:, csl], ps[0][:], bias[:, m:m + 1])
                steps.append(evac)
            return steps

        # V-proj: per-token-chunk projection (preloaded weights)
        wv_tiles = w0_tiles["v"]

        def v_proj_steps(st):
            ps = []

            def mm(i):
                if i == 0:
                    ps.append(psum_pool.tile([P, 512], F32, tag="pj",
                                             name="psv", bufs=2))
                nc.tensor.matmul(
                    ps[0][:],
                    m_cb[st // 4][i][:, (st % 4) * P:(st % 4 + 1) * P],
                    wv_tiles[i][:],
                    start=(i == 0), stop=(i == NDC - 1))
            steps = [lambda i=i, mm=mm: mm(i) for i in range(NDC)]

            def evac():
                nc.vector.tensor_scalar_mul(
                    v_aug[st][:, 0:NH, 0:64],
                    ps[0][:].rearrange("p (h d) -> p h d", h=NH),
                    vm_sb[:, st:st + 1])
                nc.gpsimd.tensor_copy(
                    v_aug[st][:, 0:NH, 64:65], vm8_sb[:, st, :].unsqueeze(2))
            steps.append(evac)
            return steps

        def v_proj(st):
            for step in v_proj_steps(st):
                step()

        wo_tiles = [[None] * NMT for _ in range(D // P)]

        def load_wo(m):
            blk = w_pool.tile([P, NMT, P], F16, tag="wo", name="wot", bufs=8)
            nc.sync.dma_start(out=blk[:], in_=wo_t[m])
            for i in range(NMT):
                wo_tiles[m][i] = blk[:, i, :]

        def o_proj_steps(m, jb):
            ps = []

            def mm(i):
                if i == 0:
                    ps.append(psum_pool.tile([P, 512], F32, tag="pj",
                                             name="pso", bufs=2))
                nc.tensor.matmul(
                    ps[0][:], wo_tiles[m][i][:],
                    attn[i][:, jb * 512:(jb + 1) * 512],
                    start=(i == 0), stop=(i == NMT - 1))
            steps = [lambda i=i, mm=mm: mm(i) for i in range(NMT)]

            def evac():
                ev = o_pool.tile([P, 512], F16, tag="ev")
                nc.vector.tensor_scalar_add(ev[:], ps[0][:], bo_sb[:, m:m + 1])
                nc.sync.dma_start(
                    out=out_t[m * P:(m + 1) * P, jb * 512:(jb + 1) * 512],
                    in_=ev[:])
            steps.append(evac)
            return steps

        # O-proj for the LAST query column: attn[0..2] partials accumulate
        # into SBUF while B(15) still runs; only the i=3 matmul + add + DMA
        # remain after the final normalize, cutting the serial tail.
        o3acc = [None] * (D // P)
        o3blk = [None]

        def o3_partial_steps(m):
            ps = []

            def mm(i):
                if i == 0:
                    if o3blk[0] is None:
                        # reuses x0's 8KB (dead after q_half(0,3))
                        o3blk[0] = x_pool.tile([P, NDC, 512], F16, tag="x0",
                                               name="o3acc", bufs=1)
                    ps.append(psum_pool.tile([P, 512], F32, tag="pj",
                                             name="pso3", bufs=2))
                nc.tensor.matmul(
                    ps[0][:], wo_tiles[m][i][:], attn[i][:, 1536:2048],
                    start=(i == 0), stop=(i == 2))
            steps = [lambda i=i, mm=mm: mm(i) for i in range(3)]

            def evac():
                o3acc[m] = o3blk[0][:, m, :]
                nc.vector.tensor_scalar_add(o3acc[m], ps[0][:],
                                            bo_sb[:, m:m + 1])
            steps.append(evac)
            return steps

        def o3_final(m):
            ps = psum_pool.tile([P, 512], F32, tag="pj", name="pso3f", bufs=2)
            nc.tensor.matmul(ps[:], wo_tiles[m][3][:], attn[3][:, 1536:2048],
                             start=True, stop=True)
            ev = o_pool.tile([P, 512], F16, tag="ev")
            nc.vector.tensor_add(ev[:], ps[:], o3acc[m])
            nc.sync.dma_start(out=out_t[m * P:(m + 1) * P, 1536:2048],
                              in_=ev[:])

        # Q weights for all head-pairs preloaded (tiny); halves emitted
        # on demand at loop tops
        wq_all = [w0_tiles["q"]]
        for m in (1, 2, 3):
            blk = w_pool.tile([P, NDC, P], F16, tag=f"wq{m}", name="wqm",
                              bufs=1)
            nc.sync.dma_start(out=blk[:], in_=wq_t[m])
            wq_all.append([blk[:, i, :] for i in range(NDC)])
        for cb in (2, 3):
            x_cb[cb] = load_blk(cb, x_t, x_pool, f"x{cb}", False)

        # ---- attention pipeline ----
        esbuf = {}
        avbuf = {}

        def emit_A(g, k2):
            mt, qb = divmod(g, 4)
            qsl = slice(qb * 512, (qb + 1) * 512)
            ka, kb = 2 * k2, 2 * k2 + 1
            lg_e = psum_pool.tile([P, 1024], F32, tag="lg", name="lg_e",
                                  bufs=2)
            lg_o = psum_pool.tile([P, 1024], F32, tag="lg", name="lg_o",
                                  bufs=2)
            # same-row-group matmuls adjacent: each PE array-config switch
            # (h0<->h64<->full) costs ~86ns of issue stall, so h0,h0,h64,h64
            # is two switches cheaper than h0,h64,h0,h64
            for half, kc in ((0, ka), (1, kb)):
                nc.tensor.matmul(
                    lg_e[:, half * 512:(half + 1) * 512],
                    kT[mt][0:64, kc * P:(kc + 1) * P],
                    qT[mt][0:64, qsl], start=True, stop=True)
            for half, kc in ((0, ka), (1, kb)):
                nc.tensor.matmul(
                    lg_o[:, half * 512:(half + 1) * 512],
                    kT[mt][64:128, kc * P:(kc + 1) * P],
                    qT[mt][64:128, qsl], start=True, stop=True)
            es_e = e_pool.tile([P, 1024], F16, tag="es", bufs=15)
            nc.scalar.activation(es_e[:], lg_e[:], AF.Exp, scale=0.125)
            es_o = e_pool.tile([P, 1024], F16, tag="es", bufs=15)
            nc.scalar.activation(es_o[:], lg_o[:], AF.Exp, scale=0.125)
            esbuf[(g, k2)] = (es_e, es_o)

        def emit_B(g, k2):
            mt, qb = divmod(g, 4)
            he, ho = 2 * mt, 2 * mt + 1
            if k2 == 0:
                avbuf[g] = (
                    psum_pool.tile([P, 512], F32, tag="av_e", name="av_e"),
                    psum_pool.tile([P, 512], F32, tag="av_o", name="av_o"))
            av_e, av_o = avbuf[g]

            es_e, es_o = esbuf.pop((g, k2))
            ka, kb = 2 * k2, 2 * k2 + 1
            for half, kc in ((0, ka), (1, kb)):
                va = v_aug[kc][:].rearrange("p h d -> p (h d)")
                nc.tensor.matmul(
                    av_e[0:65, :], va[:, 65 * he:65 * he + 65],
                    es_e[:, half * 512:(half + 1) * 512],
                    start=(k2 == 0 and half == 0),
                    stop=(k2 == NKC // 2 - 1 and half == 1))
            for half, kc in ((0, ka), (1, kb)):
                va = v_aug[kc][:].rearrange("p h d -> p (h d)")
                nc.tensor.matmul(
                    av_o[0:65, :], va[:, 65 * ho:65 * ho + 65],
                    es_o[:, half * 512:(half + 1) * 512],
                    start=(k2 == 0 and half == 0),
                    stop=(k2 == NKC // 2 - 1 and half == 1))


        def emit_norm(g):
            mt, qb = divmod(g, 4)
            qsl = slice(qb * 512, (qb + 1) * 512)
            av_e, av_o = avbuf.pop(g)
            # both heads' denominators in one chain: one reciprocal, one
            # partition_broadcast (each Pool broadcast costs ~1us + drain)
            tmp = n_pool.tile([1, 3072], F32, tag="dn", bufs=1)
            dn, r0, r0b = (tmp[:, 0:1024], tmp[:, 1024:2048],
                           tmp[:, 2048:3072])
            bc = n_pool.tile([P, 1024], F32, tag="bc", bufs=1)
            nc.vector.tensor_copy(dn[:, 0:512], av_e[64:65, :])
            nc.vector.tensor_copy(dn[:, 512:1024], av_o[64:65, :])
            # custom-DVE op; the tensor_copy after it (same DVE FIFO)
            # bridges its result into tracked dependencies for gpsimd
            nc.vector.reciprocal_approx_fast(out=r0[:], in_=dn[:])
            nc.vector.tensor_copy(r0b[:], r0[:])
            nc.gpsimd.partition_broadcast(bc[:], r0b[:])
            nc.vector.tensor_mul(
                attn[mt][0:64, qsl], av_e[0:64, :], bc[0:64, 0:512])
            nc.vector.tensor_mul(
                attn[mt][64:128, qsl], av_o[0:64, :], bc[64:128, 512:1024])

        # ---- deadline-ordered unit scheduler ----
        # Emission slot t: A(g) occupies slots [SA(g), SA(g)+8) where
        # SA(0)=0, SA(1)=8, SA(g)=16+8(g-2) for g>=2 (A(g) is emitted two
        # phases ahead of B(g), which shares slots with A(g+2)). Every
        # projection is chopped into ~0.3us units with a deadline slot;
        # the drain interleaves a few units per slot so no PE bulge ever
        # exceeds the exp-cadence slack (the lg ring buffers only one k2).
        units = deque()

        def add_units(steps, dl):
            n = len(steps)
            for i, s in enumerate(steps):
                units.append((dl - (n - 1 - i) // 3, s))

        def drain(t, eager=2):
            n = 0
            while units and (units[0][0] <= t or n < eager):
                if units[0][0] > t:
                    n += 1
                units.popleft()[1]()

        def sa(g):
            return 16 + 8 * (g - 2)

        # g = mt*4 + qb (mt slowest): kT[m] is first needed by A(4m) at
        # slot sa(4m), so K1/K2/K3 projections spread across the whole
        # timeline instead of bunching in slots 15-26. V is needed from
        # B(0) (slots 16-23) and stays early. All units queued in
        # ascending-deadline order (the drain is FIFO).
        k0 = kq_proj_steps(wk_t, m_cb, kT[0], bk_sb, 0, w_tiles=w0_tiles["k"])
        q0 = kq_proj_steps(wq_t, x_cb, qT[0], bq_sb, 0, w_tiles=w0_tiles["q"],
                           halves=[0])
        vs = [v_proj_steps(st) for st in range(NKC)]
        k1 = kq_proj_steps(wk_t, m_cb, kT[1], bk_sb, 1)
        k2s = kq_proj_steps(wk_t, m_cb, kT[2], bk_sb, 2)
        k3s = kq_proj_steps(wk_t, m_cb, kT[3], bk_sb, 3)

        def q_half(mt_, qb_):
            return kq_proj_steps(wq_t, x_cb, qT[mt_], bq_sb, mt_,
                                 w_tiles=wq_all[mt_], halves=[qb_])

        sched = [(0, k0[0:9]), (0, q0), (1, k0[9:18]),
                 (3, k0[18:27]), (5, k0[27:36])]
        for st in range(NKC):
            sched.append((3 + (st * 11) // 16, vs[st]))
        for qb_ in (1, 2, 3):
            sched.append((8 * qb_ - 2, q_half(0, qb_)))
        for m_, ks in ((1, k1), (2, k2s), (3, k3s)):
            base = sa(4 * m_)
            sched.append((base - 14, ks[0:10]))
            sched.append((base - 11, ks[10:19]))
            sched.append((base - 8, ks[19:28]))
            sched.append((base - 5, ks[28:37]))
        for g in range(4, NG):
            mt_, qb_ = divmod(g, 4)
            sched.append((sa(g) - 5, q_half(mt_, qb_)))
        for dl, steps in sorted(sched, key=lambda p: p[0]):
            add_units(steps, dl)

        for k2 in range(NKC // 2):
            drain(k2)
            emit_A(0, k2)
        for k2 in range(NKC // 2):
            drain(8 + k2)
            emit_A(1, k2)
        for g in range(NG):
            for k2 in range(NKC // 2):
                drain(sa(g + 2) + k2)
                emit_B(g, k2)
                if g + 2 < NG:
                    emit_A(g + 2, k2)
            emit_norm(g)
            mt, qb = divmod(g, 4)
            if mt == NMT - 1 and qb < 3:
                t_now = sa(g + 2)
                if qb == 2:
                    for m in range(D // P):
                        add_units(o3_partial_steps(m), t_now + 4 + m)
                for m in range(D // P):
                    if qb == 0:
                        units.append((t_now + 8 + m // 2,
                                      lambda m=m: load_wo(m)))
                    add_units(o_proj_steps(m, qb), t_now + 12 + m)

        # ---- drain remaining units, then finish the last column ----
        while units:
            units.popleft()[1]()
        for m in range(D // P):
            o3_final(m)

    nc.compile()
    return nc


def _prep_inputs(x, memory, mask, wq, bq, wk, bk, wv, bv, wo, bo):
    f = np.float32
    h = np.float16
    wqT = np.ascontiguousarray(wq.T, dtype=f)
    wkT = np.ascontiguousarray(wk.T, dtype=f)
    wvT = np.ascontiguousarray(wv.T, dtype=f)
    woT = np.ascontiguousarray(wo.T, dtype=f)
    bo_eff = (bo.astype(f) + wo.astype(f) @ bv.astype(f))
    zeros_bo = np.zeros_like(bo_eff)
    in_maps = []
    for c in range(NCORES):
        b, g = divmod(c, 2)
        sl = slice(g * OD, (g + 1) * OD)
        bo_c = bo_eff if g == 0 else zeros_bo
        vm = np.where(mask[b], np.float32(0.0), np.float32(1.0)).astype(f)
        vm_s = np.ascontiguousarray(vm.reshape(NKC, P).T)      # [P, NKC]
        vm8 = np.repeat(vm_s.astype(h)[:, :, None], NH, axis=2)  # [P,NKC,NH]
        # pack as SBUF images: [partition, free...] per-partition contiguous
        def img_blk(a):        # [D, S] -> [4, P, NDC, 512]
            return np.ascontiguousarray(
                a.reshape(NDC, P, 4, 512).transpose(2, 1, 0, 3))

        def img_w(a):          # [D, OD] -> [NMT, P, NDC, P]
            return np.ascontiguousarray(
                a.reshape(NDC, P, NMT, P).transpose(2, 1, 0, 3))

        in_maps.append({
            "x_t": img_blk(np.asarray(x[b].T, dtype=h)),
            "mem_t": img_blk(np.asarray(memory[b].T, dtype=h)),
            "wq_t": img_w(wqT[:, sl].astype(h)),
            "wk_t": img_w(wkT[:, sl].astype(h)),
            "wv_t": np.ascontiguousarray(
                wvT[:, sl].astype(h).reshape(NDC, P, OD).transpose(1, 0, 2)),
            "wo_t": np.ascontiguousarray(
                woT[sl, :].astype(h).reshape(NMT, P, D // P, P)
                .transpose(2, 1, 0, 3)),
            "bq_s": np.ascontiguousarray(bq[sl].astype(f).reshape(OD // P, P).T),
            "bk_s": np.ascontiguousarray(bk[sl].astype(f).reshape(OD // P, P).T),
            "bo_s": np.ascontiguousarray(bo_c.reshape(D // P, P).T),
            "vmask": vm_s,
            "vmask8": np.ascontiguousarray(vm8.reshape(P, NKC * NH)),
        })
    return in_maps


def kernel(x, memory, mask, wq, bq, wk, bk, wv, bv, wo, bo, **run_kwargs):
    x = np.asarray(x, dtype=np.float32)
    memory = np.asarray(memory, dtype=np.float32)
    mask = np.asarray(mask)
    if "nc" not in _cache:
        _cache["nc"] = _build()
    nc = _cache["nc"]
    in_maps = _prep_inputs(x, memory, mask, wq, bq, wk, bk, wv, bv, wo, bo)
    res = run_bass_kernel_spmd(nc, in_maps, list(range(NCORES)), **run_kwargs)
    out = np.empty((B, S, D), dtype=np.float32)
    for b in range(B):
        part = (res.results[2 * b]["out_t"].astype(np.float32)
                + res.results[2 * b + 1]["out_t"].astype(np.float32))
        out[b] = part.T
    if run_kwargs:
        _cache["last_results"] = res
    return out



# revision 28
# speedup vs baseline: 1.0591x; 1.0308x over previous
"""Multi-head cross-attention (B=4, S=2048, D=1024, H=16) on 8 Trainium2 cores.

Sharding: hybrid data/tensor parallel. Core c handles batch b = c//2 and
head-group g = c%2 (8 of the 16 heads, i.e. 512 of the 1024 q/k/v dims).
Each core computes a partial out-projection over its 512 attention dims;
the host sums the two partials per batch.

Design (v3):
- ACT engine runs ONLY exp; its ~285us busy is the kernel floor. The
  key-padding mask is applied by zeroing masked keys' V rows and ones
  column (exactly equivalent to -inf logits), so one biasless exp spans
  two key chunks ([128,1024]).
- S=K.T@Q contracts over head_dim=64: issued as PE row-tiled pairs
  (tile_position (0,0)/(64,0)) emitted adjacently so both heads stream
  concurrently on the two array halves.
- Per query block (512 queries) the work is two phases: A = S+exp for
  all 16 key chunks (es kept in a 16-deep SBUF ring), B = the 32 AV
  accumulation matmuls. A(g+1) is emitted interleaved into B(g) so the
  exp stream never waits on AV/normalize; softmax normalize runs off
  the critical path during the next block's A phase.
- All projections (K1-3, Q1-3, V, O) are pumped as small filler batches
  inside A/B so the PE stays dense (HAM stays at 2.4GHz) and projection
  time hides entirely under the exp-bound attention span.
- PSUM: lg_e(2) lg_o(2) av_e(1) av_o(1) pj(2) = 8 banks.

bv is folded into bo on the host (softmax rows sum to 1).
"""

import numpy as np

import concourse.bacc as bacc
import concourse.mybir as mybir
from concourse import tile
from concourse.bass_utils import run_bass_kernel_spmd

F32 = mybir.dt.float32
F16 = mybir.dt.float16
AF = mybir.ActivationFunctionType

B, S, D = 4, 2048, 1024
H, HD = 16, 64
NCORES = 8
NH = 8          # heads per core
OD = NH * HD    # 512 attention dims per core
P = 128
NDC = D // P    # 8 d-chunks
NKC = S // P    # 16 key chunks
NMT = OD // P   # 4 head-pairs
NG = NMT * 4    # 16 query-block groups

_cache = {}


def _build():
    from collections import deque
    from contextlib import ExitStack

    nc = bacc.Bacc(None, target_bir_lowering=False, debug=False)

    # All bulk inputs are packed host-side as exact SBUF images
    # ([partition, free...] with per-partition-contiguous free bytes) so
    # each block loads with one dma_start and 8KB-run descriptors.
    x_t = nc.dram_tensor("x_t", [4, P, NDC, 512], F16,
                         kind="ExternalInput").ap()
    mem_t = nc.dram_tensor("mem_t", [4, P, NDC, 512], F16,
                           kind="ExternalInput").ap()
    wq_t = nc.dram_tensor("wq_t", [NMT, P, NDC, P], F16,
                          kind="ExternalInput").ap()
    wk_t = nc.dram_tensor("wk_t", [NMT, P, NDC, P], F16,
                          kind="ExternalInput").ap()
    wv_t = nc.dram_tensor("wv_t", [P, NDC, OD], F16,
                          kind="ExternalInput").ap()
    wo_t = nc.dram_tensor("wo_t", [D // P, P, NMT, P], F16,
                          kind="ExternalInput").ap()
    bq_s = nc.dram_tensor("bq_s", [P, OD // P], F32, kind="ExternalInput").ap()
    bk_s = nc.dram_tensor("bk_s", [P, OD // P], F32, kind="ExternalInput").ap()
    bo_s = nc.dram_tensor("bo_s", [P, D // P], F32, kind="ExternalInput").ap()
    vmask = nc.dram_tensor("vmask", [P, NKC], F32, kind="ExternalInput").ap()
    vmask8 = nc.dram_tensor("vmask8", [P, NKC * NH], F16,
                            kind="ExternalInput").ap()
    out_t = nc.dram_tensor("out_t", [D, S], F16, kind="ExternalOutput").ap()

    with tile.TileContext(nc) as tc, ExitStack() as ctx:
        q_pool = ctx.enter_context(tc.tile_pool(name="qt", bufs=1))
        k_pool = ctx.enter_context(tc.tile_pool(name="kt", bufs=1))
        v_pool = ctx.enter_context(tc.tile_pool(name="va", bufs=1))
        a_pool = ctx.enter_context(tc.tile_pool(name="at", bufs=1))
        c_pool = ctx.enter_context(tc.tile_pool(name="cst", bufs=1))
        w_pool = ctx.enter_context(tc.tile_pool(name="wt", bufs=10))
        e_pool = ctx.enter_context(tc.tile_pool(name="es", bufs=15))
        n_pool = ctx.enter_context(tc.tile_pool(name="nrm", bufs=2))
        o_pool = ctx.enter_context(tc.tile_pool(name="ev", bufs=2))
        psum_pool = ctx.enter_context(tc.tile_pool(name="ps", bufs=1, space="PSUM"))
        m_pool = ctx.enter_context(tc.tile_pool(name="mm", bufs=8))
        x_pool = ctx.enter_context(tc.tile_pool(name="xx", bufs=8))

        # ---- constants ----
        bq_sb = c_pool.tile([P, OD // P], F32, tag="bq")
        bk_sb = c_pool.tile([P, OD // P], F32, tag="bk")
        bo_sb = c_pool.tile([P, D // P], F32, tag="bo")
        vm_sb = c_pool.tile([P, NKC], F32, tag="vm")
        vm8_sb = c_pool.tile([P, NKC, NH], F16, tag="vm8")
        nc.sync.dma_start(out=bq_sb[:], in_=bq_s[:])
        nc.sync.dma_start(out=bk_sb[:], in_=bk_s[:])
        nc.sync.dma_start(out=bo_sb[:], in_=bo_s[:])
        nc.sync.dma_start(out=vm_sb[:], in_=vmask[:])
        nc.sync.dma_start(
            out=vm8_sb[:], in_=vmask8.rearrange("p (s h) -> p s h", h=NH))

        # ---- weight DMAs first (small, needed first), then bulk inputs:
        # memory on both queues (K0/V gate on it), then x (Q0 gates on it).
        # Each block is one SBUF-image dma_start; block 0 of m/x is split
        # across two queues so the first K/Q matmuls can start earliest.
        wk0_blk = w_pool.tile([P, NDC, P], F16, tag="wk0", name="wk0", bufs=1)
        nc.sync.dma_start(out=wk0_blk[:], in_=wk_t[0])
        wq0_blk = w_pool.tile([P, NDC, P], F16, tag="wq0", name="wq0", bufs=1)
        nc.gpsimd.dma_start(out=wq0_blk[:], in_=wq_t[0])
        w0_tiles = {
            "k": [wk0_blk[:, i, :] for i in range(NDC)],
            "q": [wq0_blk[:, i, :] for i in range(NDC)],
        }
        m_cb = [None] * 4
        x_cb = [None] * 4

        QS = [nc.sync, nc.scalar]

        def load_blk(cb, src, pool, tag, pieces):
            # pieces > 1: finer semaphore granularity so the first matmuls
            # gate on a 256KB piece instead of the whole 1MB block
            t = pool.tile([P, NDC, 512], F16, tag=tag, name=tag, bufs=1)
            npc = NDC // pieces
            for j in range(pieces):
                QS[j % 2].dma_start(
                    out=t[:, j * npc:(j + 1) * npc, :],
                    in_=src[cb, :, j * npc:(j + 1) * npc, :])
            return [t[:, i, :] for i in range(NDC)]

        m_cb[0] = load_blk(0, mem_t, m_pool, "m0", 4)
        x_cb[0] = load_blk(0, x_t, x_pool, "x0", 4)
        wv_blk = w_pool.tile([P, NDC, OD], F16, tag="wv", name="wvt", bufs=1)
        nc.gpsimd.dma_start(out=wv_blk[:], in_=wv_t[:])
        w0_tiles["v"] = [wv_blk[:, i, :] for i in range(NDC)]
        for cb in (1, 2, 3):
            m_cb[cb] = load_blk(cb, mem_t, m_pool, f"m{cb}", 2)
        x_cb[1] = load_blk(1, x_t, x_pool, "x1", 2)

        # ---- persistent tiles ----
        qT = [q_pool.tile([P, S], F16, tag=f"q{m}", name=f"q{m}")
              for m in range(NMT)]
        # kT packs a head pair: partitions 0:64 = head 2m, 64:128 = head 2m+1
        kT = [k_pool.tile([P, S], F16, tag=f"k{m}", name=f"k{m}")
              for m in range(NMT)]
        v_aug = [v_pool.tile([P, NH, 65], F16, tag=f"v{st}", name=f"v{st}")
                 for st in range(NKC)]
        attn = [a_pool.tile([P, S], F16, tag=f"a{m}", name=f"a{m}")
                for m in range(NMT)]

        def kq_proj_steps(wc, src_cb, dst, bias, m, w_tiles=None,
                          halves=range(4)):
            if w_tiles is None:
                w_tiles = []

                def load_w():
                    blk = w_pool.tile([P, NDC, P], F16, tag="w", name="wkq",
                                      bufs=2)
                    nc.sync.dma_start(out=blk[:], in_=wc[m])
                    w_tiles.extend(blk[:, i, :] for i in range(NDC))
                steps = [load_w]
            else:
                steps = []
            for half in halves:
                csl = slice(half * 512, (half + 1) * 512)
                ps = []

                def mm(i, ps=ps, half=half):
                    if i == 0:
                        ps.append(psum_pool.tile([P, 512], F32, tag="pj",
                                                 name="pskq", bufs=2))
                    nc.tensor.matmul(
                        ps[0][:], w_tiles[i][:], src_cb[half][i][:],
                        start=(i == 0), stop=(i == NDC - 1))
                for i in range(NDC):
                    steps.append(lambda i=i, mm=mm: mm(i))

                def evac(ps=ps, csl=csl):
                    nc.vector.tensor_scalar_add(
                        dst[:, csl], ps[0][:], bias[:, m:m + 1])
                steps.append(evac)
            return steps

        # V-proj: per-token-chunk projection (preloaded weights)
        wv_tiles = w0_tiles["v"]

        def v_proj_steps(st):
            ps = []

            def mm(i):
                if i == 0:
                    ps.append(psum_pool.tile([P, 512], F32, tag="pj",
                                             name="psv", bufs=2))
                nc.tensor.matmul(
                    ps[0][:],
                    m_cb[st // 4][i][:, (st % 4) * P:(st % 4 + 1) * P],
                    wv_tiles[i][:],
                    start=(i == 0), stop=(i == NDC - 1))
            steps = [lambda i=i, mm=mm: mm(i) for i in range(NDC)]

            def evac():
                nc.vector.tensor_scalar_mul(
                    v_aug[st][:, 0:NH, 0:64],
                    ps[0][:].rearrange("p (h d) -> p h d", h=NH),
                    vm_sb[:, st:st + 1])
                nc.gpsimd.tensor_copy(
                    v_aug[st][:, 0:NH, 64:65], vm8_sb[:, st, :].unsqueeze(2))
            steps.append(evac)
            return steps

        def v_proj(st):
            for step in v_proj_steps(st):
                step()

        wo_tiles = [[None] * NMT for _ in range(D // P)]

        def load_wo(m):
            blk = w_pool.tile([P, NMT, P], F16, tag="wo", name="wot", bufs=8)
            nc.sync.dma_start(out=blk[:], in_=wo_t[m])
            for i in range(NMT):
                wo_tiles[m][i] = blk[:, i, :]

        def o_proj_steps(m, jb):
            ps = []

            def mm(i):
                if i == 0:
                    ps.append(psum_pool.tile([P, 512], F32, tag="pj",
                                             name="pso", bufs=2))
                nc.tensor.matmul(
                    ps[0][:], wo_tiles[m][i][:],
                    attn[i][:, jb * 512:(jb + 1) * 512],
                    start=(i == 0), stop=(i == NMT - 1))
            steps = [lambda i=i, mm=mm: mm(i) for i in range(NMT)]

            def evac():
                ev = o_pool.tile([P, 512], F16, tag="ev")
                nc.vector.tensor_scalar_add(ev[:], ps[0][:], bo_sb[:, m:m + 1])
                nc.sync.dma_start(
                    out=out_t[m * P:(m + 1) * P, jb * 512:(jb + 1) * 512],
                    in_=ev[:])
            steps.append(evac)
            return steps

        # O-proj for the LAST query column: attn[0..2] partials accumulate
        # into SBUF while B(15) still runs; only the i=3 matmul + add + DMA
        # remain after the final normalize, cutting the serial tail.
        o3acc = [None] * (D // P)
        o3blk = [None]

        def o3_partial_steps(m):
            ps = []

            def mm(i):
                if i == 0:
                    if o3blk[0] is None:
                        # reuses x0's 8KB (dead after q_half(0,3))
                        o3blk[0] = x_pool.tile([P, NDC, 512], F16, tag="x0",
                                               name="o3acc", bufs=1)
                    ps.append(psum_pool.tile([P, 512], F32, tag="pj",
                                             name="pso3", bufs=2))
                nc.tensor.matmul(
                    ps[0][:], wo_tiles[m][i][:], attn[i][:, 1536:2048],
                    start=(i == 0), stop=(i == 2))
            steps = [lambda i=i, mm=mm: mm(i) for i in range(3)]

            def evac():
                o3acc[m] = o3blk[0][:, m, :]
                nc.vector.tensor_scalar_add(o3acc[m], ps[0][:],
                                            bo_sb[:, m:m + 1])
            steps.append(evac)
            return steps

        def o3_final(m):
            ps = psum_pool.tile([P, 512], F32, tag="pj", name="pso3f", bufs=2)
            nc.tensor.matmul(ps[:], wo_tiles[m][3][:], attn[3][:, 1536:2048],
                             start=True, stop=True)
            ev = o_pool.tile([P, 512], F16, tag="ev")
            nc.vector.tensor_add(ev[:], ps[:], o3acc[m])
            nc.sync.dma_start(out=out_t[m * P:(m + 1) * P, 1536:2048],
                              in_=ev[:])

        # Q weights for all head-pairs preloaded (tiny); halves emitted
        # on demand at loop tops
        wq_all = [w0_tiles["q"]]
        for m in (1, 2, 3):
            blk = w_pool.tile([P, NDC, P], F16, tag=f"wq{m}", name="wqm",
                              bufs=1)
            nc.sync.dma_start(out=blk[:], in_=wq_t[m])
            wq_all.append([blk[:, i, :] for i in range(NDC)])
        for cb in (2, 3):
            x_cb[cb] = load_blk(cb, x_t, x_pool, f"x{cb}", 2)

        # ---- attention pipeline ----
        esbuf = {}
        avbuf = {}

        def emit_A(g, k2):
            qb, mt = divmod(g, 4)
            qsl = slice(qb * 512, (qb + 1) * 512)
            ka, kb = 2 * k2, 2 * k2 + 1
            lg_e = psum_pool.tile([P, 1024], F32, tag="lg", name="lg_e",
                                  bufs=2)
            lg_o = psum_pool.tile([P, 1024], F32, tag="lg", name="lg_o",
                                  bufs=2)
            # same-row-group matmuls adjacent: each PE array-config switch
            # (h0<->h64<->full) costs ~86ns of issue stall, so h0,h0,h64,h64
            # is two switches cheaper than h0,h64,h0,h64
            for half, kc in ((0, ka), (1, kb)):
                nc.tensor.matmul(
                    lg_e[:, half * 512:(half + 1) * 512],
                    kT[mt][0:64, kc * P:(kc + 1) * P],
                    qT[mt][0:64, qsl], start=True, stop=True)
            for half, kc in ((0, ka), (1, kb)):
                nc.tensor.matmul(
                    lg_o[:, half * 512:(half + 1) * 512],
                    kT[mt][64:128, kc * P:(kc + 1) * P],
                    qT[mt][64:128, qsl], start=True, stop=True)
            es_e = e_pool.tile([P, 1024], F16, tag="es", bufs=15)
            nc.scalar.activation(es_e[:], lg_e[:], AF.Exp, scale=0.125)
            es_o = e_pool.tile([P, 1024], F16, tag="es", bufs=15)
            nc.scalar.activation(es_o[:], lg_o[:], AF.Exp, scale=0.125)
            esbuf[(g, k2)] = (es_e, es_o)

        def emit_B(g, k2):
            qb, mt = divmod(g, 4)
            he, ho = 2 * mt, 2 * mt + 1
            if k2 == 0:
                avbuf[g] = (
                    psum_pool.tile([P, 512], F32, tag="av_e", name="av_e"),
                    psum_pool.tile([P, 512], F32, tag="av_o", name="av_o"))
            av_e, av_o = avbuf[g]

            es_e, es_o = esbuf.pop((g, k2))
            ka, kb = 2 * k2, 2 * k2 + 1
            for half, kc in ((0, ka), (1, kb)):
                va = v_aug[kc][:].rearrange("p h d -> p (h d)")
                nc.tensor.matmul(
                    av_e[0:65, :], va[:, 65 * he:65 * he + 65],
                    es_e[:, half * 512:(half + 1) * 512],
                    start=(k2 == 0 and half == 0),
                    stop=(k2 == NKC // 2 - 1 and half == 1))
            for half, kc in ((0, ka), (1, kb)):
                va = v_aug[kc][:].rearrange("p h d -> p (h d)")
                nc.tensor.matmul(
                    av_o[0:65, :], va[:, 65 * ho:65 * ho + 65],
                    es_o[:, half * 512:(half + 1) * 512],
                    start=(k2 == 0 and half == 0),
                    stop=(k2 == NKC // 2 - 1 and half == 1))


        def emit_norm(g):
            qb, mt = divmod(g, 4)
            qsl = slice(qb * 512, (qb + 1) * 512)
            av_e, av_o = avbuf.pop(g)
            # both heads' denominators in one chain: one reciprocal, one
            # partition_broadcast (each Pool broadcast costs ~1us + drain)
            tmp = n_pool.tile([1, 3072], F32, tag="dn", bufs=1)
            dn, r0, r0b = (tmp[:, 0:1024], tmp[:, 1024:2048],
                           tmp[:, 2048:3072])
            bc = n_pool.tile([P, 1024], F32, tag="bc", bufs=1)
            nc.vector.tensor_copy(dn[:, 0:512], av_e[64:65, :])
            nc.vector.tensor_copy(dn[:, 512:1024], av_o[64:65, :])
            # custom-DVE op; the tensor_copy after it (same DVE FIFO)
            # bridges its result into tracked dependencies for gpsimd
            nc.vector.reciprocal_approx_fast(out=r0[:], in_=dn[:])
            nc.vector.tensor_copy(r0b[:], r0[:])
            nc.gpsimd.partition_broadcast(bc[:], r0b[:])
            nc.vector.tensor_mul(
                attn[mt][0:64, qsl], av_e[0:64, :], bc[0:64, 0:512])
            nc.vector.tensor_mul(
                attn[mt][64:128, qsl], av_o[0:64, :], bc[64:128, 512:1024])

        # ---- deadline-ordered unit scheduler ----
        # Emission slot t: A(g) occupies slots [SA(g), SA(g)+8) where
        # SA(0)=0, SA(1)=8, SA(g)=16+8(g-2) for g>=2 (A(g) is emitted two
        # phases ahead of B(g), which shares slots with A(g+2)). Every
        # projection is chopped into ~0.3us units with a deadline slot;
        # the drain interleaves a few units per slot so no PE bulge ever
        # exceeds the exp-cadence slack (the lg ring buffers only one k2).
        units = deque()

        def add_units(steps, dl):
            n = len(steps)
            for i, s in enumerate(steps):
                units.append((dl - (n - 1 - i) // 3, s))

        def drain(t, eager=2):
            n = 0
            while units and (units[0][0] <= t or n < eager):
                if units[0][0] > t:
                    n += 1
                units.popleft()[1]()

        def sa(g):
            return 16 + 8 * (g - 2)

        k0 = kq_proj_steps(wk_t, m_cb, kT[0], bk_sb, 0, w_tiles=w0_tiles["k"])
        q0 = kq_proj_steps(wq_t, x_cb, qT[0], bq_sb, 0, w_tiles=w0_tiles["q"],
                           halves=[0])
        add_units(k0[0:9], 0)
        add_units(q0, 0)
        add_units(k0[9:18], 1)
        add_units(k0[18:27], 2)
        vs = [v_proj_steps(st) for st in range(NKC)]
        add_units(vs[0], 2)
        add_units(k0[27:36], 3)
        add_units(vs[1], 3)
        add_units(vs[2], 4)
        add_units(vs[3], 4)
        add_units(vs[4], 5)
        add_units(vs[5], 5)
        add_units(vs[6], 6)
        add_units(vs[7], 6)
        add_units(vs[8], 7)
        add_units(vs[9], 7)
        k1 = kq_proj_steps(wk_t, m_cb, kT[1], bk_sb, 1)
        add_units(k1[0:10], 8)
        add_units(kq_proj_steps(wq_t, x_cb, qT[1], bq_sb, 1,
                                w_tiles=wq_all[1], halves=[0]), 8)
        add_units(vs[10], 9)
        add_units(k1[10:19], 10)
        add_units(vs[11], 10)
        add_units(vs[12], 11)
        add_units(k1[19:28], 12)
        add_units(vs[13], 12)
        add_units(k1[28:37], 13)
        add_units(vs[14], 13)
        add_units(vs[15], 14)
        add_units(kq_proj_steps(wq_t, x_cb, qT[2], bq_sb, 2,
                                w_tiles=wq_all[2], halves=[0]), 14)
        k2s = kq_proj_steps(wk_t, m_cb, kT[2], bk_sb, 2)
        k3s = kq_proj_steps(wk_t, m_cb, kT[3], bk_sb, 3)
        add_units(k2s[0:10], 15)
        add_units(k2s[10:19], 16)
        add_units(k2s[19:28], 17)
        add_units(k2s[28:37], 18)
        add_units(k3s[0:10], 20)
        add_units(k3s[10:19], 22)
        add_units(kq_proj_steps(wq_t, x_cb, qT[3], bq_sb, 3,
                                w_tiles=wq_all[3], halves=[0]), 23)
        add_units(k3s[19:28], 24)
        add_units(k3s[28:37], 26)

        # all remaining Q halves, in deadline order, BEFORE any O units can
        # be queued: the drain is FIFO, so a due unit must never sit behind
        # a no-deadline unit
        for g in range(4, NG):
            qb_, mt_ = divmod(g, 4)
            add_units(kq_proj_steps(wq_t, x_cb, qT[mt_], bq_sb, mt_,
                                    w_tiles=wq_all[mt_], halves=[qb_]), sa(g))

        for k2 in range(NKC // 2):
            drain(k2)
            emit_A(0, k2)
        for k2 in range(NKC // 2):
            drain(8 + k2)
            emit_A(1, k2)
        for g in range(NG):
            for k2 in range(NKC // 2):
                drain(sa(g + 2) + k2)
                emit_B(g, k2)
                if g + 2 < NG:
                    emit_A(g + 2, k2)
            emit_norm(g)
            qb, mt = divmod(g, 4)
            if mt == NMT - 1 and qb < 3:
                for m in range(D // P):
                    if qb == 0:
                        units.append((10 ** 9, lambda m=m: load_wo(m)))
                    add_units(o_proj_steps(m, qb), 10 ** 9)
            if g == 14:
                # attn[0..2] cols 1536:2048 complete after norm(14)
                for m in range(D // P):
                    add_units(o3_partial_steps(m), 10 ** 9)

        # ---- drain remaining units, then finish the last column ----
        while units:
            units.popleft()[1]()
        for m in range(D // P):
            o3_final(m)

    nc.compile()
    return nc


def _prep_inputs(x, memory, mask, wq, bq, wk, bk, wv, bv, wo, bo):
    f = np.float32
    h = np.float16
    wqT = np.ascontiguousarray(wq.T, dtype=f)
    wkT = np.ascontiguousarray(wk.T, dtype=f)
    wvT = np.ascontiguousarray(wv.T, dtype=f)
    woT = np.ascontiguousarray(wo.T, dtype=f)
    bo_eff = (bo.astype(f) + wo.astype(f) @ bv.astype(f))
    zeros_bo = np.zeros_like(bo_eff)
    in_maps = []
    for c in range(NCORES):
        b, g = divmod(c, 2)
        sl = slice(g * OD, (g + 1) * OD)
        bo_c = bo_eff if g == 0 else zeros_bo
        vm = np.where(mask[b], np.float32(0.0), np.float32(1.0)).astype(f)
        vm_s = np.ascontiguousarray(vm.reshape(NKC, P).T)      # [P, NKC]
        vm8 = np.repeat(vm_s.astype(h)[:, :, None], NH, axis=2)  # [P,NKC,NH]
        # pack as SBUF images: [partition, free...] per-partition contiguous
        def img_blk(a):        # [D, S] -> [4, P, NDC, 512]
            return np.ascontiguousarray(
                a.reshape(NDC, P, 4, 512).transpose(2, 1, 0, 3))

        def img_w(a):          # [D, OD] -> [NMT, P, NDC, P]
            return np.ascontiguousarray(
                a.reshape(NDC, P, NMT, P).transpose(2, 1, 0, 3))

        in_maps.append({
            "x_t": img_blk(np.asarray(x[b].T, dtype=h)),
            "mem_t": img_blk(np.asarray(memory[b].T, dtype=h)),
            "wq_t": img_w(wqT[:, sl].astype(h)),
            "wk_t": img_w(wkT[:, sl].astype(h)),
            "wv_t": np.ascontiguousarray(
                wvT[:, sl].astype(h).reshape(NDC, P, OD).transpose(1, 0, 2)),
            "wo_t": np.ascontiguousarray(
                woT[sl, :].astype(h).reshape(NMT, P, D // P, P)
                .transpose(2, 1, 0, 3)),
            "bq_s": np.ascontiguousarray(bq[sl].astype(f).reshape(OD // P, P).T),
            "bk_s": np.ascontiguousarray(bk[sl].astype(f).reshape(OD // P, P).T),
            "bo_s": np.ascontiguousarray(bo_c.reshape(D // P, P).T),
            "vmask": vm_s,
            "vmask8": np.ascontiguousarray(vm8.reshape(P, NKC * NH)),
        })
    return in_maps


def kernel(x, memory, mask, wq, bq, wk, bk, wv, bv, wo, bo, **run_kwargs):
    x = np.asarray(x, dtype=np.float32)
    memory = np.asarray(memory, dtype=np.float32)
    mask = np.asarray(mask)
    if "nc" not in _cache:
        _cache["nc"] = _build()
    nc = _cache["nc"]
    in_maps = _prep_inputs(x, memory, mask, wq, bq, wk, bk, wv, bv, wo, bo)
    res = run_bass_kernel_spmd(nc, in_maps, list(range(NCORES)), **run_kwargs)
    out = np.empty((B, S, D), dtype=np.float32)
    for b in range(B):
        part = (res.results[2 * b]["out_t"].astype(np.float32)
                + res.results[2 * b + 1]["out_t"].astype(np.float32))
        out[b] = part.T
    if run_kwargs:
        _cache["last_results"] = res
    return out



# revision 36
# speedup vs baseline: 1.0639x; 1.0045x over previous
"""Multi-head cross-attention (B=4, S=2048, D=1024, H=16) on 8 Trainium2 cores.

Sharding: hybrid data/tensor parallel. Core c handles batch b = c//2 and
head-group g = c%2 (8 of the 16 heads, i.e. 512 of the 1024 q/k/v dims).
Each core computes a partial out-projection over its 512 attention dims;
the host sums the two partials per batch.

Design (v3):
- ACT engine runs ONLY exp; its ~285us busy is the kernel floor. The
  key-padding mask is applied by zeroing masked keys' V rows and ones
  column (exactly equivalent to -inf logits), so one biasless exp spans
  two key chunks ([128,1024]).
- S=K.T@Q contracts over head_dim=64: issued as PE row-tiled pairs
  (tile_position (0,0)/(64,0)) emitted adjacently so both heads stream
  concurrently on the two array halves.
- Per query block (512 queries) the work is two phases: A = S+exp for
  all 16 key chunks (es kept in a 16-deep SBUF ring), B = the 32 AV
  accumulation matmuls. A(g+1) is emitted interleaved into B(g) so the
  exp stream never waits on AV/normalize; softmax normalize runs off
  the critical path during the next block's A phase.
- All projections (K1-3, Q1-3, V, O) are pumped as small filler batches
  inside A/B so the PE stays dense (HAM stays at 2.4GHz) and projection
  time hides entirely under the exp-bound attention span.
- PSUM: lg_e(2) lg_o(2) av_e(1) av_o(1) pj(2) = 8 banks.

bv is folded into bo on the host (softmax rows sum to 1).
"""

import numpy as np

import concourse.bacc as bacc
import concourse.mybir as mybir
from concourse import tile
from concourse.bass_utils import run_bass_kernel_spmd

F32 = mybir.dt.float32
F16 = mybir.dt.float16
AF = mybir.ActivationFunctionType

B, S, D = 4, 2048, 1024
H, HD = 16, 64
NCORES = 8
NH = 8          # heads per core
OD = NH * HD    # 512 attention dims per core
P = 128
NDC = D // P    # 8 d-chunks
NKC = S // P    # 16 key chunks
NMT = OD // P   # 4 head-pairs
NG = NMT * 4    # 16 query-block groups

_cache = {}


def _build():
    from collections import deque
    from contextlib import ExitStack

    nc = bacc.Bacc(None, target_bir_lowering=False, debug=False)

    # All bulk inputs are packed host-side as exact SBUF images
    # ([partition, free...] with per-partition-contiguous free bytes) so
    # each block loads with one dma_start and 8KB-run descriptors.
    x_t = nc.dram_tensor("x_t", [4, P, NDC, 512], F16,
                         kind="ExternalInput").ap()
    mem_t = nc.dram_tensor("mem_t", [4, P, NDC, 512], F16,
                           kind="ExternalInput").ap()
    wq_t = nc.dram_tensor("wq_t", [NMT, P, NDC, P], F16,
                          kind="ExternalInput").ap()
    wk_t = nc.dram_tensor("wk_t", [NMT, P, NDC, P], F16,
                          kind="ExternalInput").ap()
    wv_t = nc.dram_tensor("wv_t", [P, NDC, OD], F16,
                          kind="ExternalInput").ap()
    wo_t = nc.dram_tensor("wo_t", [D // P, P, NMT, P], F16,
                          kind="ExternalInput").ap()
    bq_s = nc.dram_tensor("bq_s", [P, OD // P], F32, kind="ExternalInput").ap()
    bk_s = nc.dram_tensor("bk_s", [P, OD // P], F32, kind="ExternalInput").ap()
    bo_s = nc.dram_tensor("bo_s", [P, D // P], F32, kind="ExternalInput").ap()
    vmask = nc.dram_tensor("vmask", [P, NKC], F32, kind="ExternalInput").ap()
    vmask8 = nc.dram_tensor("vmask8", [P, NKC * NH], F16,
                            kind="ExternalInput").ap()
    out_t = nc.dram_tensor("out_t", [D, S], F16, kind="ExternalOutput").ap()

    with tile.TileContext(nc) as tc, ExitStack() as ctx:
        q_pool = ctx.enter_context(tc.tile_pool(name="qt", bufs=1))
        k_pool = ctx.enter_context(tc.tile_pool(name="kt", bufs=1))
        v_pool = ctx.enter_context(tc.tile_pool(name="va", bufs=1))
        a_pool = ctx.enter_context(tc.tile_pool(name="at", bufs=1))
        c_pool = ctx.enter_context(tc.tile_pool(name="cst", bufs=1))
        w_pool = ctx.enter_context(tc.tile_pool(name="wt", bufs=10))
        e_pool = ctx.enter_context(tc.tile_pool(name="es", bufs=16))
        n_pool = ctx.enter_context(tc.tile_pool(name="nrm", bufs=2))
        o_pool = ctx.enter_context(tc.tile_pool(name="ev", bufs=2))
        psum_pool = ctx.enter_context(tc.tile_pool(name="ps", bufs=1, space="PSUM"))
        m_pool = ctx.enter_context(tc.tile_pool(name="mm", bufs=8))
        x_pool = ctx.enter_context(tc.tile_pool(name="xx", bufs=8))

        # ---- weight DMAs first (small, needed first), then bulk inputs:
        # memory on both queues (K0/V gate on it), then x (Q0 gates on it).
        # Each block is one SBUF-image dma_start. Constants are loaded
        # AFTER the first-needed bulk blocks (their first use is ~slot 1).
        bq_sb = c_pool.tile([P, OD // P], F32, tag="bq")
        bk_sb = c_pool.tile([P, OD // P], F32, tag="bk")
        bo_sb = c_pool.tile([P, D // P], F32, tag="bo")
        vm_sb = c_pool.tile([P, NKC], F32, tag="vm")
        vm8_sb = c_pool.tile([P, NKC, NH], F16, tag="vm8")
        wk0_blk = w_pool.tile([P, NDC, P], F16, tag="wk0", name="wk0", bufs=1)
        nc.sync.dma_start(out=wk0_blk[:, 0:4, :], in_=wk_t[0, :, 0:4, :])
        nc.scalar.dma_start(out=wk0_blk[:, 4:8, :], in_=wk_t[0, :, 4:8, :])
        wq0_blk = w_pool.tile([P, NDC, P], F16, tag="wq0", name="wq0", bufs=1)
        nc.gpsimd.dma_start(out=wq0_blk[:], in_=wq_t[0])
        w0_tiles = {
            "k": [wk0_blk[:, i, :] for i in range(NDC)],
            "q": [wq0_blk[:, i, :] for i in range(NDC)],
        }
        m_cb = [None] * 4
        x_cb = [None] * 4

        QS = [nc.sync, nc.scalar]

        def load_blk(cb, src, pool, tag, pieces):
            # pieces > 1: finer semaphore granularity so the first matmuls
            # gate on a 256KB piece instead of the whole 1MB block
            t = pool.tile([P, NDC, 512], F16, tag=tag, name=tag, bufs=1)
            npc = NDC // pieces
            for j in range(pieces):
                QS[j % 2].dma_start(
                    out=t[:, j * npc:(j + 1) * npc, :],
                    in_=src[cb, :, j * npc:(j + 1) * npc, :])
            return [t[:, i, :] for i in range(NDC)]

        m_cb[0] = load_blk(0, mem_t, m_pool, "m0", 2)
        x_cb[0] = load_blk(0, x_t, x_pool, "x0", 2)
        wv_blk = w_pool.tile([P, NDC, OD], F16, tag="wv", name="wvt", bufs=1)
        nc.gpsimd.dma_start(out=wv_blk[:], in_=wv_t[:])
        w0_tiles["v"] = [wv_blk[:, i, :] for i in range(NDC)]
        nc.gpsimd.dma_start(out=bk_sb[:], in_=bk_s[:])
        nc.gpsimd.dma_start(out=bq_sb[:], in_=bq_s[:])
        nc.gpsimd.dma_start(out=vm_sb[:], in_=vmask[:])
        nc.gpsimd.dma_start(
            out=vm8_sb[:], in_=vmask8.rearrange("p (s h) -> p s h", h=NH))
        nc.gpsimd.dma_start(out=bo_sb[:], in_=bo_s[:])
        for cb in (1, 2, 3):
            m_cb[cb] = load_blk(cb, mem_t, m_pool, f"m{cb}", 2)
        x_cb[1] = load_blk(1, x_t, x_pool, "x1", 2)

        # ---- persistent tiles ----
        qT = [q_pool.tile([P, S], F16, tag=f"q{m}", name=f"q{m}")
              for m in range(NMT)]
        # kT packs a head pair: partitions 0:64 = head 2m, 64:128 = head 2m+1
        kT = [k_pool.tile([P, S], F16, tag=f"k{m}", name=f"k{m}")
              for m in range(NMT)]
        v_aug = [v_pool.tile([P, NH, 65], F16, tag=f"v{st}", name=f"v{st}")
                 for st in range(NKC)]
        attn = [a_pool.tile([P, S], F16, tag=f"a{m}", name=f"a{m}")
                for m in range(NMT)]

        def kq_proj_steps(wc, src_cb, dst, bias, m, w_tiles=None,
                          halves=range(4)):
            if w_tiles is None:
                w_tiles = []

                def load_w():
                    blk = w_pool.tile([P, NDC, P], F16, tag="w", name="wkq",
                                      bufs=2)
                    nc.sync.dma_start(out=blk[:], in_=wc[m])
                    w_tiles.extend(blk[:, i, :] for i in range(NDC))
                steps = [load_w]
            else:
                steps = []
            for half in halves:
                csl = slice(half * 512, (half + 1) * 512)
                ps = []

                def mm(i, ps=ps, half=half):
                    if i == 0:
                        ps.append(psum_pool.tile([P, 512], F32, tag="pj",
                                                 name="pskq", bufs=2))
                    nc.tensor.matmul(
                        ps[0][:], w_tiles[i][:], src_cb[half][i][:],
                        start=(i == 0), stop=(i == NDC - 1))
                for i in range(NDC):
                    steps.append(lambda i=i, mm=mm: mm(i))

                def evac(ps=ps, csl=csl):
                    nc.vector.tensor_scalar_add(
                        dst[:, csl], ps[0][:], bias[:, m:m + 1])
                steps.append(evac)
            return steps

        # V-proj: per-token-chunk projection (preloaded weights)
        wv_tiles = w0_tiles["v"]

        def v_proj_steps(st):
            ps = []

            def mm(i):
                if i == 0:
                    ps.append(psum_pool.tile([P, 512], F32, tag="pj",
                                             name="psv", bufs=2))
                nc.tensor.matmul(
                    ps[0][:],
                    m_cb[st // 4][i][:, (st % 4) * P:(st % 4 + 1) * P],
                    wv_tiles[i][:],
                    start=(i == 0), stop=(i == NDC - 1))
            steps = [lambda i=i, mm=mm: mm(i) for i in range(NDC)]

            def evac():
                nc.vector.tensor_scalar_mul(
                    v_aug[st][:, 0:NH, 0:64],
                    ps[0][:].rearrange("p (h d) -> p h d", h=NH),
                    vm_sb[:, st:st + 1])
                nc.gpsimd.tensor_copy(
                    v_aug[st][:, 0:NH, 64:65], vm8_sb[:, st, :].unsqueeze(2))
            steps.append(evac)
            return steps

        def v_proj(st):
            for step in v_proj_steps(st):
                step()

        # output staging: an 8-slot ring reusing m1's SBUF (dead after
        # K1/V proj). Deep ring so evac->DMA chains never stall on a
        # previous output block's transfer completing.
        ev_ring = [None, 0]

        def ev_slot():
            if ev_ring[0] is None:
                ev_ring[0] = m_pool.tile([P, NDC, 512], F16, tag="m1",
                                         name="evring", bufs=1)
            s = ev_ring[1] % NDC
            ev_ring[1] += 1
            return ev_ring[0][:, s, :]

        wo_tiles = [[None] * NMT for _ in range(D // P)]

        def load_wo(m):
            blk = w_pool.tile([P, NMT, P], F16, tag="wo", name="wot", bufs=8)
            nc.sync.dma_start(out=blk[:], in_=wo_t[m])
            for i in range(NMT):
                wo_tiles[m][i] = blk[:, i, :]

        def o_proj_steps(m, jb):
            ps = []

            def mm(i):
                if i == 0:
                    ps.append(psum_pool.tile([P, 512], F32, tag="pj",
                                             name="pso", bufs=2))
                nc.tensor.matmul(
                    ps[0][:], wo_tiles[m][i][:],
                    attn[i][:, jb * 512:(jb + 1) * 512],
                    start=(i == 0), stop=(i == NMT - 1))
            steps = [lambda i=i, mm=mm: mm(i) for i in range(NMT)]

            def evac():
                ev = ev_slot()
                nc.vector.tensor_scalar_add(ev, ps[0][:], bo_sb[:, m:m + 1])
                nc.sync.dma_start(
                    out=out_t[m * P:(m + 1) * P, jb * 512:(jb + 1) * 512],
                    in_=ev)
            steps.append(evac)
            return steps

        # O-proj for the LAST query column: attn[0..2] partials accumulate
        # into SBUF while B(15) still runs; only the i=3 matmul + add + DMA
        # remain after the final normalize, cutting the serial tail.
        o3acc = [None] * (D // P)
        o3blk = [None]

        def o3_partial_steps(m):
            ps = []

            def mm(i):
                if i == 0:
                    if o3blk[0] is None:
                        # reuses x0's 8KB (dead after q_half(0,3))
                        o3blk[0] = x_pool.tile([P, NDC, 512], F16, tag="x0",
                                               name="o3acc", bufs=1)
                    ps.append(psum_pool.tile([P, 512], F32, tag="pj",
                                             name="pso3", bufs=2))
                nc.tensor.matmul(
                    ps[0][:], wo_tiles[m][i][:], attn[i][:, 1536:2048],
                    start=(i == 0), stop=(i == 2))
            steps = [lambda i=i, mm=mm: mm(i) for i in range(3)]

            def evac():
                o3acc[m] = o3blk[0][:, m, :]
                nc.vector.tensor_scalar_add(o3acc[m], ps[0][:],
                                            bo_sb[:, m:m + 1])
            steps.append(evac)
            return steps

        def o3_final(m):
            ps = psum_pool.tile([P, 512], F32, tag="pj", name="pso3f", bufs=2)
            nc.tensor.matmul(ps[:], wo_tiles[m][3][:], attn[3][:, 1536:2048],
                             start=True, stop=True)
            ev = ev_slot()
            nc.vector.tensor_add(ev, ps[:], o3acc[m])
            QS[m % 2].dma_start(out=out_t[m * P:(m + 1) * P, 1536:2048],
                                in_=ev)

        # Q weights for all head-pairs preloaded (tiny); halves emitted
        # on demand at loop tops
        wq_all = [w0_tiles["q"]]
        for m in (1, 2, 3):
            blk = w_pool.tile([P, NDC, P], F16, tag=f"wq{m}", name="wqm",
                              bufs=1)
            nc.sync.dma_start(out=blk[:], in_=wq_t[m])
            wq_all.append([blk[:, i, :] for i in range(NDC)])
        for cb in (2, 3):
            x_cb[cb] = load_blk(cb, x_t, x_pool, f"x{cb}", 2)

        # ---- attention pipeline ----
        esbuf = {}
        avbuf = {}

        def emit_A(g, k2):
            qb, mt = divmod(g, 4)
            qsl = slice(qb * 512, (qb + 1) * 512)
            ka, kb = 2 * k2, 2 * k2 + 1
            lg_e = psum_pool.tile([P, 1024], F32, tag="lg", name="lg_e",
                                  bufs=2)
            lg_o = psum_pool.tile([P, 1024], F32, tag="lg", name="lg_o",
                                  bufs=2)
            # same-row-group matmuls adjacent: each PE array-config switch
            # (h0<->h64<->full) costs ~86ns of issue stall, so h0,h0,h64,h64
            # is two switches cheaper than h0,h64,h0,h64
            for half, kc in ((0, ka), (1, kb)):
                nc.tensor.matmul(
                    lg_e[:, half * 512:(half + 1) * 512],
                    kT[mt][0:64, kc * P:(kc + 1) * P],
                    qT[mt][0:64, qsl], start=True, stop=True)
            for half, kc in ((0, ka), (1, kb)):
                nc.tensor.matmul(
                    lg_o[:, half * 512:(half + 1) * 512],
                    kT[mt][64:128, kc * P:(kc + 1) * P],
                    qT[mt][64:128, qsl], start=True, stop=True)
            es_e = e_pool.tile([P, 1024], F16, tag="es", bufs=16)
            nc.scalar.activation(es_e[:], lg_e[:], AF.Exp, scale=0.125)
            es_o = e_pool.tile([P, 1024], F16, tag="es", bufs=16)
            nc.scalar.activation(es_o[:], lg_o[:], AF.Exp, scale=0.125)
            esbuf[(g, k2)] = (es_e, es_o)

        def emit_B(g, k2):
            qb, mt = divmod(g, 4)
            he, ho = 2 * mt, 2 * mt + 1
            if k2 == 0:
                avbuf[g] = (
                    psum_pool.tile([P, 512], F32, tag="av_e", name="av_e"),
                    psum_pool.tile([P, 512], F32, tag="av_o", name="av_o"))
            av_e, av_o = avbuf[g]

            es_e, es_o = esbuf.pop((g, k2))
            ka, kb = 2 * k2, 2 * k2 + 1
            for half, kc in ((0, ka), (1, kb)):
                va = v_aug[kc][:].rearrange("p h d -> p (h d)")
                nc.tensor.matmul(
                    av_e[0:65, :], va[:, 65 * he:65 * he + 65],
                    es_e[:, half * 512:(half + 1) * 512],
                    start=(k2 == 0 and half == 0),
                    stop=(k2 == NKC // 2 - 1 and half == 1))
            for half, kc in ((0, ka), (1, kb)):
                va = v_aug[kc][:].rearrange("p h d -> p (h d)")
                nc.tensor.matmul(
                    av_o[0:65, :], va[:, 65 * ho:65 * ho + 65],
                    es_o[:, half * 512:(half + 1) * 512],
                    start=(k2 == 0 and half == 0),
                    stop=(k2 == NKC // 2 - 1 and half == 1))


        def emit_norm(g):
            qb, mt = divmod(g, 4)
            qsl = slice(qb * 512, (qb + 1) * 512)
            av_e, av_o = avbuf.pop(g)
            # both heads' denominators in one chain: one reciprocal, one
            # partition_broadcast (each Pool broadcast costs ~1us + drain)
            tmp = n_pool.tile([1, 3072], F32, tag="dn", bufs=1)
            dn, r0, r0b = (tmp[:, 0:1024], tmp[:, 1024:2048],
                           tmp[:, 2048:3072])
            bc = n_pool.tile([P, 1024], F32, tag="bc", bufs=1)
            nc.vector.tensor_copy(dn[:, 0:512], av_e[64:65, :])
            nc.vector.tensor_copy(dn[:, 512:1024], av_o[64:65, :])
            # custom-DVE op; the tensor_copy after it (same DVE FIFO)
            # bridges its result into tracked dependencies for gpsimd
            nc.vector.reciprocal_approx_fast(out=r0[:], in_=dn[:])
            nc.vector.tensor_copy(r0b[:], r0[:])
            nc.gpsimd.partition_broadcast(bc[:], r0b[:])
            nc.vector.tensor_mul(
                attn[mt][0:64, qsl], av_e[0:64, :], bc[0:64, 0:512])
            nc.vector.tensor_mul(
                attn[mt][64:128, qsl], av_o[0:64, :], bc[64:128, 512:1024])

        # ---- deadline-ordered unit scheduler ----
        # Emission slot t: A(g) occupies slots [SA(g), SA(g)+8) where
        # SA(0)=0, SA(1)=8, SA(g)=16+8(g-2) for g>=2 (A(g) is emitted two
        # phases ahead of B(g), which shares slots with A(g+2)). Every
        # projection is chopped into ~0.3us units with a deadline slot;
        # the drain interleaves a few units per slot so no PE bulge ever
        # exceeds the exp-cadence slack (the lg ring buffers only one k2).
        units = deque()

        def add_units(steps, dl):
            n = len(steps)
            for i, s in enumerate(steps):
                units.append((dl - (n - 1 - i) // 3, s))

        def drain(t, eager=2):
            n = 0
            while units and (units[0][0] <= t or n < eager):
                if units[0][0] > t:
                    n += 1
                units.popleft()[1]()

        def sa(g):
            return 16 + 8 * (g - 2)

        k0 = kq_proj_steps(wk_t, m_cb, kT[0], bk_sb, 0, w_tiles=w0_tiles["k"])
        q0 = kq_proj_steps(wq_t, x_cb, qT[0], bq_sb, 0, w_tiles=w0_tiles["q"],
                           halves=[0])
        add_units(k0[0:9], 0)
        add_units(q0, 0)
        add_units(k0[9:18], 1)
        add_units(k0[18:27], 2)
        vs = [v_proj_steps(st) for st in range(NKC)]
        add_units(k0[27:36], 3)
        add_units(vs[0], 4)
        add_units(vs[1], 4)
        add_units(vs[2], 5)
        add_units(vs[3], 5)
        add_units(vs[4], 6)
        add_units(vs[5], 6)
        add_units(vs[6], 7)
        add_units(vs[7], 7)
        add_units(vs[8], 8)
        add_units(vs[9], 8)
        k1 = kq_proj_steps(wk_t, m_cb, kT[1], bk_sb, 1)
        add_units(k1[0:10], 8)
        add_units(kq_proj_steps(wq_t, x_cb, qT[1], bq_sb, 1,
                                w_tiles=wq_all[1], halves=[0]), 8)
        add_units(vs[10], 9)
        add_units(k1[10:19], 10)
        add_units(vs[11], 10)
        add_units(vs[12], 11)
        add_units(k1[19:28], 12)
        add_units(vs[13], 12)
        add_units(k1[28:37], 13)
        add_units(vs[14], 13)
        add_units(vs[15], 14)
        add_units(kq_proj_steps(wq_t, x_cb, qT[2], bq_sb, 2,
                                w_tiles=wq_all[2], halves=[0]), 14)
        k2s = kq_proj_steps(wk_t, m_cb, kT[2], bk_sb, 2)
        k3s = kq_proj_steps(wk_t, m_cb, kT[3], bk_sb, 3)
        add_units(k2s[0:10], 15)
        add_units(k2s[10:19], 16)
        add_units(k2s[19:28], 17)
        add_units(k2s[28:37], 18)
        add_units(k3s[0:10], 20)
        add_units(k3s[10:19], 22)
        add_units(kq_proj_steps(wq_t, x_cb, qT[3], bq_sb, 3,
                                w_tiles=wq_all[3], halves=[0]), 23)
        add_units(k3s[19:28], 24)
        add_units(k3s[28:37], 26)

        # all remaining Q halves, in deadline order, BEFORE any O units can
        # be queued: the drain is FIFO, so a due unit must never sit behind
        # a no-deadline unit
        for g in range(4, NG):
            qb_, mt_ = divmod(g, 4)
            add_units(kq_proj_steps(wq_t, x_cb, qT[mt_], bq_sb, mt_,
                                    w_tiles=wq_all[mt_], halves=[qb_]), sa(g))

        for k2 in range(NKC // 2):
            drain(k2)
            emit_A(0, k2)
        for k2 in range(NKC // 2):
            drain(8 + k2)
            emit_A(1, k2)
        for g in range(NG):
            for k2 in range(NKC // 2):
                drain(sa(g + 2) + k2)
                emit_B(g, k2)
                if g + 2 < NG:
                    emit_A(g + 2, k2)
            emit_norm(g)
            qb, mt = divmod(g, 4)
            if mt == NMT - 1 and qb < 3:
                for m in range(D // P):
                    if qb == 0:
                        units.append((10 ** 9, lambda m=m: load_wo(m)))
                    add_units(o_proj_steps(m, qb), 10 ** 9)
            if g == 14:
                # attn[0..2] cols 1536:2048 complete after norm(14)
                for m in range(D // P):
                    add_units(o3_partial_steps(m), 10 ** 9)

        # ---- drain remaining units, then finish the last column ----
        while units:
            units.popleft()[1]()
        for m in range(D // P):
            o3_final(m)

    nc.compile()
    return nc


def _prep_inputs(x, memory, mask, wq, bq, wk, bk, wv, bv, wo, bo):
    f = np.float32
    h = np.float16
    wqT = np.ascontiguousarray(wq.T, dtype=f)
    wkT = np.ascontiguousarray(wk.T, dtype=f)
    wvT = np.ascontiguousarray(wv.T, dtype=f)
    woT = np.ascontiguousarray(wo.T, dtype=f)
    bo_eff = (bo.astype(f) + wo.astype(f) @ bv.astype(f))
    zeros_bo = np.zeros_like(bo_eff)
    in_maps = []
    for c in range(NCORES):
        b, g = divmod(c, 2)
        sl = slice(g * OD, (g + 1) * OD)
        bo_c = bo_eff if g == 0 else zeros_bo
        vm = np.where(mask[b], np.float32(0.0), np.float32(1.0)).astype(f)
        vm_s = np.ascontiguousarray(vm.reshape(NKC, P).T)      # [P, NKC]
        vm8 = np.repeat(vm_s.astype(h)[:, :, None], NH, axis=2)  # [P,NKC,NH]
        # pack as SBUF images: [partition, free...] per-partition contiguous
        def img_blk(a):        # [D, S] -> [4, P, NDC, 512]
            return np.ascontiguousarray(
                a.reshape(NDC, P, 4, 512).transpose(2, 1, 0, 3))

        def img_w(a):          # [D, OD] -> [NMT, P, NDC, P]
            return np.ascontiguousarray(
                a.reshape(NDC, P, NMT, P).transpose(2, 1, 0, 3))

        in_maps.append({
            "x_t": img_blk(np.asarray(x[b].T, dtype=h)),
            "mem_t": img_blk(np.asarray(memory[b].T, dtype=h)),
            "wq_t": img_w(wqT[:, sl].astype(h)),
            "wk_t": img_w(wkT[:, sl].astype(h)),
            "wv_t": np.ascontiguousarray(
                wvT[:, sl].astype(h).reshape(NDC, P, OD).transpose(1, 0, 2)),
            "wo_t": np.ascontiguousarray(
                woT[sl, :].astype(h).reshape(NMT, P, D // P, P)
                .transpose(2, 1, 0, 3)),
            "bq_s": np.ascontiguousarray(bq[sl].astype(f).reshape(OD // P, P).T),
            "bk_s": np.ascontiguousarray(bk[sl].astype(f).reshape(OD // P, P).T),
            "bo_s": np.ascontiguousarray(bo_c.reshape(D // P, P).T),
            "vmask": vm_s,
            "vmask8": np.ascontiguousarray(vm8.reshape(P, NKC * NH)),
        })
    return in_maps


def kernel(x, memory, mask, wq, bq, wk, bk, wv, bv, wo, bo, **run_kwargs):
    x = np.asarray(x, dtype=np.float32)
    memory = np.asarray(memory, dtype=np.float32)
    mask = np.asarray(mask)
    if "nc" not in _cache:
        _cache["nc"] = _build()
    nc = _cache["nc"]
    in_maps = _prep_inputs(x, memory, mask, wq, bq, wk, bk, wv, bv, wo, bo)
    res = run_bass_kernel_spmd(nc, in_maps, list(range(NCORES)), **run_kwargs)
    out = np.empty((B, S, D), dtype=np.float32)
    for b in range(B):
        part = (res.results[2 * b]["out_t"].astype(np.float32)
                + res.results[2 * b + 1]["out_t"].astype(np.float32))
        out[b] = part.T
    if run_kwargs:
        _cache["last_results"] = res
    return out



# revision 38
# speedup vs baseline: 1.0691x; 1.0049x over previous
"""Multi-head cross-attention (B=4, S=2048, D=1024, H=16) on 8 Trainium2 cores.

Sharding: hybrid data/tensor parallel. Core c handles batch b = c//2 and
head-group g = c%2 (8 of the 16 heads, i.e. 512 of the 1024 q/k/v dims).
Each core computes a partial out-projection over its 512 attention dims;
the host sums the two partials per batch.

Design (v8):
- PE stream time (~400us: 1536 N=512 matmuls + overheads) is the
  binding cost; ACT exp busy is ~285us. The HAM power limiter claws
  back over-dense PE schedules (measured: razor-JIT filler deadlines
  tripled throttle_active and regressed 30us), so the filler schedule
  keeps the baseline's moderate-density deadline table.
- The key-padding mask is applied by zeroing masked keys' V rows and
  ones column (exactly equivalent to -inf logits), so one biasless exp
  spans two key chunks ([128,1024]).
- S=K.T@Q contracts over head_dim=64 as row-tiled pairs (tile_position
  (0,0)/(64,0)); same-row-group matmuls are emitted adjacently
  (h0,h0,h64,h64) so the PE's limited OOO can dual-issue opposite
  halves when both PSUM operands are free.
- Per query block (512 queries): A = S+exp for 16 key chunks (es in a
  16-deep SBUF ring), B = 32 AV accumulation matmuls; A(g+2) emits
  interleaved into B(g). K/Q/V/O projections drain as deadline-ordered
  filler units.
- All bulk inputs are packed host-side as SBUF images (per-partition
  contiguous) for 8KB-run DMA descriptors; constants load after the
  first-needed blocks. First matmul at ~10us.
- Tail: O-proj for the last query column pre-accumulates attn[0..2]
  partials into SBUF (x0's space) while B(15) runs; after the final
  normalize only 8x(matmul+add+DMA) remain. Output staging is an
  8-slot ring in m1's dead space so evac never waits on a prior
  output DMA.
- PSUM: lg_e(2) lg_o(2) av_e(1) av_o(1) pj(2) = 8 banks.

bv is folded into bo on the host (softmax rows sum to 1).
"""

import numpy as np

import concourse.bacc as bacc
import concourse.mybir as mybir
from concourse import tile
from concourse.bass_utils import run_bass_kernel_spmd

F32 = mybir.dt.float32
F16 = mybir.dt.float16
AF = mybir.ActivationFunctionType

B, S, D = 4, 2048, 1024
H, HD = 16, 64
NCORES = 8
NH = 8          # heads per core
OD = NH * HD    # 512 attention dims per core
P = 128
NDC = D // P    # 8 d-chunks
NKC = S // P    # 16 key chunks
NMT = OD // P   # 4 head-pairs
NG = NMT * 4    # 16 query-block groups

_cache = {}


def _build():
    from collections import deque
    from contextlib import ExitStack

    nc = bacc.Bacc(None, target_bir_lowering=False, debug=False)

    # All bulk inputs are packed host-side as exact SBUF images
    # ([partition, free...] with per-partition-contiguous free bytes) so
    # each block loads with one dma_start and 8KB-run descriptors.
    x_t = nc.dram_tensor("x_t", [4, P, NDC, 512], F16,
                         kind="ExternalInput").ap()
    mem_t = nc.dram_tensor("mem_t", [4, P, NDC, 512], F16,
                           kind="ExternalInput").ap()
    wq_t = nc.dram_tensor("wq_t", [NMT, P, NDC, P], F16,
                          kind="ExternalInput").ap()
    wk_t = nc.dram_tensor("wk_t", [NMT, P, NDC, P], F16,
                          kind="ExternalInput").ap()
    wv_t = nc.dram_tensor("wv_t", [P, NDC, OD], F16,
                          kind="ExternalInput").ap()
    wo_t = nc.dram_tensor("wo_t", [D // P, P, NMT, P], F16,
                          kind="ExternalInput").ap()
    bq_s = nc.dram_tensor("bq_s", [P, OD // P], F32, kind="ExternalInput").ap()
    bk_s = nc.dram_tensor("bk_s", [P, OD // P], F32, kind="ExternalInput").ap()
    bo_s = nc.dram_tensor("bo_s", [P, D // P], F32, kind="ExternalInput").ap()
    vmask = nc.dram_tensor("vmask", [P, NKC], F32, kind="ExternalInput").ap()
    vmask8 = nc.dram_tensor("vmask8", [P, NKC * NH], F16,
                            kind="ExternalInput").ap()
    out_t = nc.dram_tensor("out_t", [D, S], F16, kind="ExternalOutput").ap()

    with tile.TileContext(nc) as tc, ExitStack() as ctx:
        q_pool = ctx.enter_context(tc.tile_pool(name="qt", bufs=1))
        k_pool = ctx.enter_context(tc.tile_pool(name="kt", bufs=1))
        v_pool = ctx.enter_context(tc.tile_pool(name="va", bufs=1))
        a_pool = ctx.enter_context(tc.tile_pool(name="at", bufs=1))
        c_pool = ctx.enter_context(tc.tile_pool(name="cst", bufs=1))
        w_pool = ctx.enter_context(tc.tile_pool(name="wt", bufs=10))
        e_pool = ctx.enter_context(tc.tile_pool(name="es", bufs=16))
        n_pool = ctx.enter_context(tc.tile_pool(name="nrm", bufs=2))
        o_pool = ctx.enter_context(tc.tile_pool(name="ev", bufs=2))
        psum_pool = ctx.enter_context(tc.tile_pool(name="ps", bufs=1, space="PSUM"))
        m_pool = ctx.enter_context(tc.tile_pool(name="mm", bufs=8))
        x_pool = ctx.enter_context(tc.tile_pool(name="xx", bufs=8))

        # ---- weight DMAs first (small, needed first), then bulk inputs:
        # memory on both queues (K0/V gate on it), then x (Q0 gates on it).
        # Each block is one SBUF-image dma_start. Constants are loaded
        # AFTER the first-needed bulk blocks (their first use is ~slot 1).
        bq_sb = c_pool.tile([P, OD // P], F32, tag="bq")
        bk_sb = c_pool.tile([P, OD // P], F32, tag="bk")
        bo_sb = c_pool.tile([P, D // P], F32, tag="bo")
        vm_sb = c_pool.tile([P, NKC], F32, tag="vm")
        vm8_sb = c_pool.tile([P, NKC, NH], F16, tag="vm8")
        wk0_blk = w_pool.tile([P, NDC, P], F16, tag="wk0", name="wk0", bufs=1)
        nc.sync.dma_start(out=wk0_blk[:, 0:4, :], in_=wk_t[0, :, 0:4, :])
        nc.scalar.dma_start(out=wk0_blk[:, 4:8, :], in_=wk_t[0, :, 4:8, :])
        wq0_blk = w_pool.tile([P, NDC, P], F16, tag="wq0", name="wq0", bufs=1)
        nc.gpsimd.dma_start(out=wq0_blk[:], in_=wq_t[0])
        w0_tiles = {
            "k": [wk0_blk[:, i, :] for i in range(NDC)],
            "q": [wq0_blk[:, i, :] for i in range(NDC)],
        }
        m_cb = [None] * 4
        x_cb = [None] * 4

        QS = [nc.sync, nc.scalar]

        def load_blk(cb, src, pool, tag, pieces):
            # pieces > 1: finer semaphore granularity so the first matmuls
            # gate on a 256KB piece instead of the whole 1MB block
            t = pool.tile([P, NDC, 512], F16, tag=tag, name=tag, bufs=1)
            npc = NDC // pieces
            for j in range(pieces):
                QS[j % 2].dma_start(
                    out=t[:, j * npc:(j + 1) * npc, :],
                    in_=src[cb, :, j * npc:(j + 1) * npc, :])
            return [t[:, i, :] for i in range(NDC)]

        m_cb[0] = load_blk(0, mem_t, m_pool, "m0", 2)
        x_cb[0] = load_blk(0, x_t, x_pool, "x0", 2)
        nc.gpsimd.dma_start(out=bk_sb[:], in_=bk_s[:])
        nc.gpsimd.dma_start(out=bq_sb[:], in_=bq_s[:])
        wv_blk = w_pool.tile([P, NDC, OD], F16, tag="wv", name="wvt", bufs=1)
        nc.gpsimd.dma_start(out=wv_blk[:], in_=wv_t[:])
        w0_tiles["v"] = [wv_blk[:, i, :] for i in range(NDC)]
        nc.gpsimd.dma_start(out=vm_sb[:], in_=vmask[:])
        nc.gpsimd.dma_start(
            out=vm8_sb[:], in_=vmask8.rearrange("p (s h) -> p s h", h=NH))
        nc.gpsimd.dma_start(out=bo_sb[:], in_=bo_s[:])
        for cb in (1, 2, 3):
            m_cb[cb] = load_blk(cb, mem_t, m_pool, f"m{cb}", 2)
        x_cb[1] = load_blk(1, x_t, x_pool, "x1", 2)

        # ---- persistent tiles ----
        qT = [q_pool.tile([P, S], F16, tag=f"q{m}", name=f"q{m}")
              for m in range(NMT)]
        # kT packs a head pair: partitions 0:64 = head 2m, 64:128 = head 2m+1
        kT = [k_pool.tile([P, S], F16, tag=f"k{m}", name=f"k{m}")
              for m in range(NMT)]
        v_aug = [v_pool.tile([P, NH, 65], F16, tag=f"v{st}", name=f"v{st}")
                 for st in range(NKC)]
        attn = [a_pool.tile([P, S], F16, tag=f"a{m}", name=f"a{m}")
                for m in range(NMT)]

        def kq_proj_steps(wc, src_cb, dst, bias, m, w_tiles=None,
                          halves=range(4)):
            if w_tiles is None:
                w_tiles = []

                def load_w():
                    blk = w_pool.tile([P, NDC, P], F16, tag="w", name="wkq",
                                      bufs=2)
                    nc.sync.dma_start(out=blk[:], in_=wc[m])
                    w_tiles.extend(blk[:, i, :] for i in range(NDC))
                steps = [load_w]
            else:
                steps = []
            for half in halves:
                csl = slice(half * 512, (half + 1) * 512)
                ps = []

                def mm(i, ps=ps, half=half):
                    if i == 0:
                        ps.append(psum_pool.tile([P, 512], F32, tag="pj",
                                                 name="pskq", bufs=2))
                    nc.tensor.matmul(
                        ps[0][:], w_tiles[i][:], src_cb[half][i][:],
                        start=(i == 0), stop=(i == NDC - 1))
                for i in range(NDC):
                    steps.append(lambda i=i, mm=mm: mm(i))

                def evac(ps=ps, csl=csl):
                    nc.vector.tensor_scalar_add(
                        dst[:, csl], ps[0][:], bias[:, m:m + 1])
                steps.append(evac)
            return steps

        # V-proj: per-token-chunk projection (preloaded weights)
        wv_tiles = w0_tiles["v"]

        def v_proj_steps(st):
            ps = []

            def mm(i):
                if i == 0:
                    ps.append(psum_pool.tile([P, 512], F32, tag="pj",
                                             name="psv", bufs=2))
                nc.tensor.matmul(
                    ps[0][:],
                    m_cb[st // 4][i][:, (st % 4) * P:(st % 4 + 1) * P],
                    wv_tiles[i][:],
                    start=(i == 0), stop=(i == NDC - 1))
            steps = [lambda i=i, mm=mm: mm(i) for i in range(NDC)]

            def evac():
                nc.vector.tensor_scalar_mul(
                    v_aug[st][:, 0:NH, 0:64],
                    ps[0][:].rearrange("p (h d) -> p h d", h=NH),
                    vm_sb[:, st:st + 1])
                nc.gpsimd.tensor_copy(
                    v_aug[st][:, 0:NH, 64:65], vm8_sb[:, st, :].unsqueeze(2))
            steps.append(evac)
            return steps

        def v_proj(st):
            for step in v_proj_steps(st):
                step()

        # output staging: an 8-slot ring reusing m1's SBUF (dead after
        # K1/V proj). Deep ring so evac->DMA chains never stall on a
        # previous output block's transfer completing.
        ev_ring = [None, 0]

        def ev_slot():
            if ev_ring[0] is None:
                ev_ring[0] = m_pool.tile([P, NDC, 512], F16, tag="m1",
                                         name="evring", bufs=1)
            s = ev_ring[1] % NDC
            ev_ring[1] += 1
            return ev_ring[0][:, s, :]

        wo_tiles = [[None] * NMT for _ in range(D // P)]

        def load_wo(m):
            blk = w_pool.tile([P, NMT, P], F16, tag="wo", name="wot", bufs=8)
            nc.sync.dma_start(out=blk[:], in_=wo_t[m])
            for i in range(NMT):
                wo_tiles[m][i] = blk[:, i, :]

        def o_proj_steps(m, jb):
            ps = []

            def mm(i):
                if i == 0:
                    ps.append(psum_pool.tile([P, 512], F32, tag="pj",
                                             name="pso", bufs=2))
                nc.tensor.matmul(
                    ps[0][:], wo_tiles[m][i][:],
                    attn[i][:, jb * 512:(jb + 1) * 512],
                    start=(i == 0), stop=(i == NMT - 1))
            steps = [lambda i=i, mm=mm: mm(i) for i in range(NMT)]

            def evac():
                ev = ev_slot()
                nc.vector.tensor_scalar_add(ev, ps[0][:], bo_sb[:, m:m + 1])
                nc.sync.dma_start(
                    out=out_t[m * P:(m + 1) * P, jb * 512:(jb + 1) * 512],
                    in_=ev)
            steps.append(evac)
            return steps

        # O-proj for the LAST query column: attn[0..2] partials accumulate
        # into SBUF while B(15) still runs; only the i=3 matmul + add + DMA
        # remain after the final normalize, cutting the serial tail.
        o3acc = [None] * (D // P)
        o3blk = [None]

        def o3_partial_steps(m):
            ps = []

            def mm(i):
                if i == 0:
                    if o3blk[0] is None:
                        # reuses x0's 8KB (dead after q_half(0,3))
                        o3blk[0] = x_pool.tile([P, NDC, 512], F16, tag="x0",
                                               name="o3acc", bufs=1)
                    ps.append(psum_pool.tile([P, 512], F32, tag="pj",
                                             name="pso3", bufs=2))
                nc.tensor.matmul(
                    ps[0][:], wo_tiles[m][i][:], attn[i][:, 1536:2048],
                    start=(i == 0), stop=(i == 2))
            steps = [lambda i=i, mm=mm: mm(i) for i in range(3)]

            def evac():
                o3acc[m] = o3blk[0][:, m, :]
                nc.vector.tensor_scalar_add(o3acc[m], ps[0][:],
                                            bo_sb[:, m:m + 1])
            steps.append(evac)
            return steps

        def o3_final(m):
            ps = psum_pool.tile([P, 512], F32, tag="pj", name="pso3f", bufs=2)
            nc.tensor.matmul(ps[:], wo_tiles[m][3][:], attn[3][:, 1536:2048],
                             start=True, stop=True)
            ev = ev_slot()
            nc.vector.tensor_add(ev, ps[:], o3acc[m])
            QS[m % 2].dma_start(out=out_t[m * P:(m + 1) * P, 1536:2048],
                                in_=ev)

        # Q weights for all head-pairs preloaded (tiny); halves emitted
        # on demand at loop tops
        wq_all = [w0_tiles["q"]]
        for m in (1, 2, 3):
            blk = w_pool.tile([P, NDC, P], F16, tag=f"wq{m}", name="wqm",
                              bufs=1)
            nc.sync.dma_start(out=blk[:], in_=wq_t[m])
            wq_all.append([blk[:, i, :] for i in range(NDC)])
        for cb in (2, 3):
            x_cb[cb] = load_blk(cb, x_t, x_pool, f"x{cb}", 2)

        # ---- attention pipeline ----
        esbuf = {}
        avbuf = {}

        def emit_A(g, k2):
            qb, mt = divmod(g, 4)
            qsl = slice(qb * 512, (qb + 1) * 512)
            ka, kb = 2 * k2, 2 * k2 + 1
            lg_e = psum_pool.tile([P, 1024], F32, tag="lg", name="lg_e",
                                  bufs=2)
            lg_o = psum_pool.tile([P, 1024], F32, tag="lg", name="lg_o",
                                  bufs=2)
            # same-row-group matmuls adjacent: each PE array-config switch
            # (h0<->h64<->full) costs ~86ns of issue stall, so h0,h0,h64,h64
            # is two switches cheaper than h0,h64,h0,h64
            for half, kc in ((0, ka), (1, kb)):
                nc.tensor.matmul(
                    lg_e[:, half * 512:(half + 1) * 512],
                    kT[mt][0:64, kc * P:(kc + 1) * P],
                    qT[mt][0:64, qsl], start=True, stop=True)
            for half, kc in ((0, ka), (1, kb)):
                nc.tensor.matmul(
                    lg_o[:, half * 512:(half + 1) * 512],
                    kT[mt][64:128, kc * P:(kc + 1) * P],
                    qT[mt][64:128, qsl], start=True, stop=True)
            es_e = e_pool.tile([P, 1024], F16, tag="es", bufs=16)
            nc.scalar.activation(es_e[:], lg_e[:], AF.Exp, scale=0.125)
            es_o = e_pool.tile([P, 1024], F16, tag="es", bufs=16)
            nc.scalar.activation(es_o[:], lg_o[:], AF.Exp, scale=0.125)
            esbuf[(g, k2)] = (es_e, es_o)

        def emit_B(g, k2):
            qb, mt = divmod(g, 4)
            he, ho = 2 * mt, 2 * mt + 1
            if k2 == 0:
                avbuf[g] = (
                    psum_pool.tile([P, 512], F32, tag="av_e", name="av_e"),
                    psum_pool.tile([P, 512], F32, tag="av_o", name="av_o"))
            av_e, av_o = avbuf[g]

            es_e, es_o = esbuf.pop((g, k2))
            ka, kb = 2 * k2, 2 * k2 + 1
            for half, kc in ((0, ka), (1, kb)):
                va = v_aug[kc][:].rearrange("p h d -> p (h d)")
                nc.tensor.matmul(
                    av_e[0:65, :], va[:, 65 * he:65 * he + 65],
                    es_e[:, half * 512:(half + 1) * 512],
                    start=(k2 == 0 and half == 0),
                    stop=(k2 == NKC // 2 - 1 and half == 1))
            for half, kc in ((0, ka), (1, kb)):
                va = v_aug[kc][:].rearrange("p h d -> p (h d)")
                nc.tensor.matmul(
                    av_o[0:65, :], va[:, 65 * ho:65 * ho + 65],
                    es_o[:, half * 512:(half + 1) * 512],
                    start=(k2 == 0 and half == 0),
                    stop=(k2 == NKC // 2 - 1 and half == 1))


        def emit_norm(g):
            qb, mt = divmod(g, 4)
            qsl = slice(qb * 512, (qb + 1) * 512)
            av_e, av_o = avbuf.pop(g)
            # both heads' denominators in one chain: one reciprocal, one
            # partition_broadcast (each Pool broadcast costs ~1us + drain)
            tmp = n_pool.tile([1, 3072], F32, tag="dn", bufs=1)
            dn, r0, r0b = (tmp[:, 0:1024], tmp[:, 1024:2048],
                           tmp[:, 2048:3072])
            bc = n_pool.tile([P, 1024], F32, tag="bc", bufs=1)
            nc.vector.tensor_copy(dn[:, 0:512], av_e[64:65, :])
            nc.vector.tensor_copy(dn[:, 512:1024], av_o[64:65, :])
            # custom-DVE op; the tensor_copy after it (same DVE FIFO)
            # bridges its result into tracked dependencies for gpsimd
            nc.vector.reciprocal_approx_fast(out=r0[:], in_=dn[:])
            nc.vector.tensor_copy(r0b[:], r0[:])
            nc.gpsimd.partition_broadcast(bc[:], r0b[:])
            nc.vector.tensor_mul(
                attn[mt][0:64, qsl], av_e[0:64, :], bc[0:64, 0:512])
            nc.vector.tensor_mul(
                attn[mt][64:128, qsl], av_o[0:64, :], bc[64:128, 512:1024])

        # ---- deadline-ordered unit scheduler ----
        # Emission slot t: A(g) occupies slots [SA(g), SA(g)+8) where
        # SA(0)=0, SA(1)=8, SA(g)=16+8(g-2) for g>=2 (A(g) is emitted two
        # phases ahead of B(g), which shares slots with A(g+2)). Every
        # projection is chopped into ~0.3us units with a deadline slot;
        # the drain interleaves a few units per slot so no PE bulge ever
        # exceeds the exp-cadence slack (the lg ring buffers only one k2).
        units = deque()

        def add_units(steps, dl):
            n = len(steps)
            for i, s in enumerate(steps):
                units.append((dl - (n - 1 - i) // 3, s))

        def drain(t, eager=2):
            n = 0
            while units and (units[0][0] <= t or n < eager):
                if units[0][0] > t:
                    n += 1
                units.popleft()[1]()

        def sa(g):
            return 16 + 8 * (g - 2)

        k0 = kq_proj_steps(wk_t, m_cb, kT[0], bk_sb, 0, w_tiles=w0_tiles["k"])
        q0 = kq_proj_steps(wq_t, x_cb, qT[0], bq_sb, 0, w_tiles=w0_tiles["q"],
                           halves=[0])
        add_units(k0[0:9], 0)
        add_units(q0, 0)
        add_units(k0[9:18], 1)
        add_units(k0[18:27], 2)
        vs = [v_proj_steps(st) for st in range(NKC)]
        add_units(k0[27:36], 3)
        add_units(vs[0], 4)
        add_units(vs[1], 4)
        add_units(vs[2], 5)
        add_units(vs[3], 5)
        add_units(vs[4], 6)
        add_units(vs[5], 6)
        add_units(vs[6], 7)
        add_units(vs[7], 7)
        add_units(vs[8], 8)
        add_units(vs[9], 8)
        k1 = kq_proj_steps(wk_t, m_cb, kT[1], bk_sb, 1)
        add_units(k1[0:10], 8)
        add_units(kq_proj_steps(wq_t, x_cb, qT[1], bq_sb, 1,
                                w_tiles=wq_all[1], halves=[0]), 8)
        add_units(vs[10], 9)
        add_units(k1[10:19], 10)
        add_units(vs[11], 10)
        add_units(vs[12], 11)
        add_units(k1[19:28], 12)
        add_units(vs[13], 12)
        add_units(k1[28:37], 13)
        add_units(vs[14], 13)
        add_units(vs[15], 14)
        add_units(kq_proj_steps(wq_t, x_cb, qT[2], bq_sb, 2,
                                w_tiles=wq_all[2], halves=[0]), 14)
        k2s = kq_proj_steps(wk_t, m_cb, kT[2], bk_sb, 2)
        k3s = kq_proj_steps(wk_t, m_cb, kT[3], bk_sb, 3)
        add_units(k2s[0:10], 15)
        add_units(k2s[10:19], 16)
        add_units(k2s[19:28], 17)
        add_units(k2s[28:37], 18)
        add_units(k3s[0:10], 20)
        add_units(k3s[10:19], 22)
        add_units(kq_proj_steps(wq_t, x_cb, qT[3], bq_sb, 3,
                                w_tiles=wq_all[3], halves=[0]), 23)
        add_units(k3s[19:28], 24)
        add_units(k3s[28:37], 26)

        # all remaining Q halves, in deadline order, BEFORE any O units can
        # be queued: the drain is FIFO, so a due unit must never sit behind
        # a no-deadline unit
        for g in range(4, NG):
            qb_, mt_ = divmod(g, 4)
            add_units(kq_proj_steps(wq_t, x_cb, qT[mt_], bq_sb, mt_,
                                    w_tiles=wq_all[mt_], halves=[qb_]), sa(g))

        for k2 in range(NKC // 2):
            drain(k2)
            emit_A(0, k2)
        for k2 in range(NKC // 2):
            drain(8 + k2)
            emit_A(1, k2)
        for g in range(NG):
            for k2 in range(NKC // 2):
                drain(sa(g + 2) + k2)
                emit_B(g, k2)
                if g + 2 < NG:
                    emit_A(g + 2, k2)
            emit_norm(g)
            qb, mt = divmod(g, 4)
            if mt == NMT - 1 and qb < 3:
                for m in range(D // P):
                    if qb == 0:
                        units.append((10 ** 9, lambda m=m: load_wo(m)))
                    add_units(o_proj_steps(m, qb), 10 ** 9)
            if g == 14:
                # attn[0..2] cols 1536:2048 complete after norm(14)
                for m in range(D // P):
                    add_units(o3_partial_steps(m), 10 ** 9)

        # ---- drain remaining units, then finish the last column ----
        while units:
            units.popleft()[1]()
        for m in range(D // P):
            o3_final(m)

    nc.compile()
    return nc


def _prep_inputs(x, memory, mask, wq, bq, wk, bk, wv, bv, wo, bo):
    f = np.float32
    h = np.float16
    wqT = np.ascontiguousarray(wq.T, dtype=f)
    wkT = np.ascontiguousarray(wk.T, dtype=f)
    wvT = np.ascontiguousarray(wv.T, dtype=f)
    woT = np.ascontiguousarray(wo.T, dtype=f)
    bo_eff = (bo.astype(f) + wo.astype(f) @ bv.astype(f))
    zeros_bo = np.zeros_like(bo_eff)
    in_maps = []
    for c in range(NCORES):
        b, g = divmod(c, 2)
        sl = slice(g * OD, (g + 1) * OD)
        bo_c = bo_eff if g == 0 else zeros_bo
        vm = np.where(mask[b], np.float32(0.0), np.float32(1.0)).astype(f)
        vm_s = np.ascontiguousarray(vm.reshape(NKC, P).T)      # [P, NKC]
        vm8 = np.repeat(vm_s.astype(h)[:, :, None], NH, axis=2)  # [P,NKC,NH]
        # pack as SBUF images: [partition, free...] per-partition contiguous
        def img_blk(a):        # [D, S] -> [4, P, NDC, 512]
            return np.ascontiguousarray(
                a.reshape(NDC, P, 4, 512).transpose(2, 1, 0, 3))

        def img_w(a):          # [D, OD] -> [NMT, P, NDC, P]
            return np.ascontiguousarray(
                a.reshape(NDC, P, NMT, P).transpose(2, 1, 0, 3))

        in_maps.append({
            "x_t": img_blk(np.asarray(x[b].T, dtype=h)),
            "mem_t": img_blk(np.asarray(memory[b].T, dtype=h)),
            "wq_t": img_w(wqT[:, sl].astype(h)),
            "wk_t": img_w(wkT[:, sl].astype(h)),
            "wv_t": np.ascontiguousarray(
                wvT[:, sl].astype(h).reshape(NDC, P, OD).transpose(1, 0, 2)),
            "wo_t": np.ascontiguousarray(
                woT[sl, :].astype(h).reshape(NMT, P, D // P, P)
                .transpose(2, 1, 0, 3)),
            "bq_s": np.ascontiguousarray(bq[sl].astype(f).reshape(OD // P, P).T),
            "bk_s": np.ascontiguousarray(bk[sl].astype(f).reshape(OD // P, P).T),
            "bo_s": np.ascontiguousarray(bo_c.reshape(D // P, P).T),
            "vmask": vm_s,
            "vmask8": np.ascontiguousarray(vm8.reshape(P, NKC * NH)),
        })
    return in_maps


def kernel(x, memory, mask, wq, bq, wk, bk, wv, bv, wo, bo, **run_kwargs):
    x = np.asarray(x, dtype=np.float32)
    memory = np.asarray(memory, dtype=np.float32)
    mask = np.asarray(mask)
    if "nc" not in _cache:
        _cache["nc"] = _build()
    nc = _cache["nc"]
    in_maps = _prep_inputs(x, memory, mask, wq, bq, wk, bk, wv, bv, wo, bo)
    res = run_bass_kernel_spmd(nc, in_maps, list(range(NCORES)), **run_kwargs)
    out = np.empty((B, S, D), dtype=np.float32)
    for b in range(B):
        part = (res.results[2 * b]["out_t"].astype(np.float32)
                + res.results[2 * b + 1]["out_t"].astype(np.float32))
        out[b] = part.T
    if run_kwargs:
        _cache["last_results"] = res
    return out

